# revision 1
# baseline (speedup 1.0000x reference)
"""Trainium2 Bass kernel for per-token grouped attention (GQA-style).

Computation (per token t):
    q = x @ Wq.T + bq ; k = x @ Wk.T + bk ; v = x @ Wv.T + bv     (D=2048)
    reshape to (G=16 groups, d=128); scores = q_g . k_h / sqrt(d) (16x16)
    att = softmax(scores, axis=h); out = att @ v  -> (G*d,)

Sharding: data-parallel over the B*T = 16384 tokens across 8 cores
(2048 tokens/core).  Everything on-device is feature-major ("transposed")
so that the PE contracts over the partition axis; the host transposes x
on the way in and the output on the way out.

Device program (per core, SPMD):
  Phase 1 (projections): qT/kT/vT = W.T-tiles @ xT, bf16 matmuls with
    fp32 PSUM accumulation, bias added during the PSUM->SBUF copy (ACT),
    results spilled to DRAM in token-tile-major layout.
  Phase 2 (attention): tokens processed in blocks of 8; one 128x128
    matmul computes all 64 pairwise 16x16 score tiles of an 8-token
    block (only the 8 diagonal tiles are kept - masked softmax), then a
    block-diagonal trick turns att @ v into another 128x128 matmul after
    two PE transposes.  Output is written feature-major and transposed
    back on the host.
"""

import os
import numpy as np
import ml_dtypes

import concourse.bass as bass
import concourse.tile as tile
from concourse import bacc, mybir
from concourse.bass_utils import run_bass_kernel_spmd

F32 = mybir.dt.float32
BF16 = mybir.dt.bfloat16
AF = mybir.ActivationFunctionType
ALU = mybir.AluOpType

P = 128          # SBUF partitions
D = 2048         # model dim
G = 16           # groups
DG = 128         # per-group dim
N_CORES = 8
TC = 2048        # tokens per core
NCHUNK = 4      # phase-1 token chunks
CH = TC // NCHUNK          # 512
NTILE = 8       # phase-2 token tiles
TT = TC // NTILE           # 256
NSB = TT // 32  # super-blocks per tile (4 blocks of 8 tokens each) = 8
KT = D // P      # 16 contraction tiles
MT = D // P      # 16 output-feature tiles


def _emit(nc, tc, ctx):
    # ---- DRAM I/O -------------------------------------------------------
    xT = nc.dram_tensor("xT", [D, TC], BF16, kind="ExternalInput").ap()
    wT = {
        p: nc.dram_tensor(f"w{p}T", [D, D], BF16, kind="ExternalInput").ap()
        for p in "qkv"
    }
    b_dram = {
        p: nc.dram_tensor(f"b{p}", [P, G], F32, kind="ExternalInput").ap()
        for p in "qkv"
    }
    m01_dram = nc.dram_tensor("m01", [P, 4, P], F32, kind="ExternalInput").ap()
    ident_dram = nc.dram_tensor("ident", [P, P], BF16, kind="ExternalInput").ap()
    outT = nc.dram_tensor("outT", [D, TC], F32, kind="ExternalOutput").ap()

    # ---- pools ----------------------------------------------------------
    singles = ctx.enter_context(tc.tile_pool(name="singles", bufs=1))
    xpool = ctx.enter_context(tc.tile_pool(name="xpool", bufs=2))
    wpool = ctx.enter_context(tc.tile_pool(name="wpool", bufs=6))
    pp_ps = ctx.enter_context(tc.tile_pool(name="pp_ps", bufs=2, space="PSUM"))
    asmp = ctx.enter_context(tc.tile_pool(name="asmp", bufs=2))

    qkvp = ctx.enter_context(tc.tile_pool(name="qkvp", bufs=2))
    otp = ctx.enter_context(tc.tile_pool(name="otp", bufs=2))
    smallp = ctx.enter_context(tc.tile_pool(name="smallp", bufs=2))
    attp = ctx.enter_context(tc.tile_pool(name="attp", bufs=2))
    trp = ctx.enter_context(tc.tile_pool(name="trp", bufs=2))
    ps_s = ctx.enter_context(tc.tile_pool(name="ps_s", bufs=2, space="PSUM"))
    ps_att = ctx.enter_context(tc.tile_pool(name="ps_att", bufs=1, space="PSUM"))
    ps_vt = ctx.enter_context(tc.tile_pool(name="ps_vt", bufs=1, space="PSUM"))
    ps_o = ctx.enter_context(tc.tile_pool(name="ps_o", bufs=2, space="PSUM"))

    # ---- constants ------------------------------------------------------
    m01_sb = singles.tile([P, 4, P], F32, tag="m01", name="m01")
    nc.sync.dma_start(out=m01_sb[:], in_=m01_dram[:])
    ident_sb = singles.tile([P, P], BF16, tag="ident", name="ident")
    nc.sync.dma_start(out=ident_sb[:], in_=ident_dram[:])
    bias_sb = {}
    for p in "qkv":
        bias_sb[p] = singles.tile([P, G], F32, tag=f"bias{p}", name=f"bias{p}")
        nc.sync.dma_start(out=bias_sb[p][:], in_=b_dram[p][:])

    # assembled q/k/v chunk tiles stay resident in SBUF (block-interleaved
    # [dd, block, g, s]); attention reads them directly - no DRAM round-trip.
    chunk_asm = {}

    # DRAM views
    xT_v = xT.rearrange("(k p) t -> p k t", p=P)          # [P, KT, TC]
    wT_v = {p: wT[p].rearrange("(k p) o -> p k o", p=P) for p in "qkv"}
    outT_v = outT.rearrange("(g p) t -> p g t", p=P)       # [P, G, TC]

    # ---- attention emission pieces -------------------------------------
    # Each token tile yields: a prologue (loads), 8 A-pieces (scores MMs +
    # softmax chain) and 8 B-pieces (v-transpose + att@v + out scatter), and
    # an epilogue (store).  Pieces are pumped one-per-m-group into the
    # projection emission of the NEXT chunk so DVE/ACT softmax work hides
    # under projection matmuls and the PE never waits on it.
    def make_tile_pieces(t):
        st = {}
        c, half = t // (CH // TT), t % (CH // TT)
        nb = TT // 8

        def prologue():
            st["ot"] = otp.tile([P, G, TT], F32, tag="ot", name="ot")
            st["att"] = {}

        def piece_a(sb):
            q2f = chunk_asm[c]["q"].rearrange("p b g s -> p (b g s)")
            k2f = chunk_asm[c]["k"].rearrange("p b g s -> p (b g s)")
            s_ps = ps_s.tile([P, 4, P], F32, tag="s", name="s")
            for j in range(4):
                b = half * nb + sb * 4 + j
                sl = slice(b * P, (b + 1) * P)
                nc.tensor.matmul(s_ps[:, j, :], lhsT=q2f[:, sl], rhs=k2f[:, sl],
                                 start=True, stop=True)
            # masked softmax over the 16-wide diagonal tiles
            e = smallp.tile([P, 4, P], F32, tag="e", name="e")
            nc.scalar.activation(out=e[:], in_=s_ps[:], func=AF.Exp)
            nc.vector.tensor_tensor(out=e[:], in0=e[:], in1=m01_sb[:],
                                    op=ALU.mult)
            sums = smallp.tile([P, 4], F32, tag="sums", name="sums")
            nc.vector.tensor_reduce(out=sums[:], in_=e[:],
                                    axis=mybir.AxisListType.X, op=ALU.add)
            rs = smallp.tile([P, 4], F32, tag="rs", name="rs")
            nc.vector.reciprocal(out=rs[:], in_=sums[:])
            att = attp.tile([P, 4, P], BF16, tag="att", name="att")
            for j in range(4):
                nc.vector.tensor_scalar_mul(att[:, j, :], e[:, j, :],
                                            rs[:, j:j + 1])
            a_ps = ps_att.tile([P, 4, P], BF16, tag="a", name="a")
            for j in range(4):
                nc.tensor.transpose(a_ps[:, j, :], att[:, j, :], ident_sb[:])
            attT = trp.tile([P, 4, P], BF16, tag="attT", name="attT")
            nc.scalar.copy(out=attT[:], in_=a_ps[:])
            st["att"][sb] = attT

        def piece_b(sb):
            t0 = sb * 32
            v2f = chunk_asm[c]["v"].rearrange("p b g s -> p (b g s)")
            attT = st["att"].pop(sb)
            # transpose v blocks: [d, (s,h)] -> [(s,h), d]
            v_ps = ps_vt.tile([P, 4, P], BF16, tag="v", name="v")
            for j in range(4):
                b = half * nb + sb * 4 + j
                nc.tensor.transpose(v_ps[:, j, :], v2f[:, b * P:(b + 1) * P],
                                    ident_sb[:])
            vT = trp.tile([P, 4, P], BF16, tag="vT", name="vT")
            nc.scalar.copy(out=vT[:], in_=v_ps[:])
            # att @ v -> out^T block [d, (s,g)]
            o_ps = ps_o.tile([P, 4, P], F32, tag="o", name="o")
            for j in range(4):
                nc.tensor.matmul(o_ps[:, j, :], lhsT=vT[:, j, :],
                                 rhs=attT[:, j, :], start=True, stop=True)
            dst = st["ot"][:, :, t0:t0 + 32].rearrange("p g (j s) -> p g j s", j=4)
            src = o_ps[:].rearrange("p j (g s) -> p g j s", g=16)
            nc.vector.tensor_copy(out=dst, in_=src)

        def epilogue():
            nc.gpsimd.dma_start(out=outT_v[:, :, t * TT:(t + 1) * TT],
                                in_=st["ot"][:])

        a = [lambda sb=sb: piece_a(sb) for sb in range(NSB)]
        b = [lambda sb=sb: piece_b(sb) for sb in range(NSB)]
        return prologue, a, b, epilogue

    def chunk_pieces(c):
        """Pieces for the two token tiles computed in chunk c, B lagging A."""
        t0, t1 = 2 * c, 2 * c + 1
        p0, a0, b0, e0 = make_tile_pieces(t0)
        p1, a1, b1, e1 = make_tile_pieces(t1)
        aa = a0 + a1
        bb = b0 + b1
        out = [p0, p1]
        lag = 2
        for i in range(len(aa) + lag):
            if i < len(aa):
                out.append(aa[i])
            if i >= lag:
                out.append(bb[i - lag])
        out += [e0, e1]
        return out

    def last_chunk_pieces(c):
        """Split: [qk prologues + A pieces] pumped into this chunk's own
        v-projection; [v loads + B pieces + epilogues] drain at the end."""
        t0, t1 = 2 * c, 2 * c + 1
        p0, a0, b0, e0 = make_tile_pieces(t0)
        p1, a1, b1, e1 = make_tile_pieces(t1)
        pre = [p0, p1] + a0 + a1
        post = b0 + b1 + [e0, e1]
        return pre, post

    # ---- phase 1: projections with attention pieces pumped in ----------
    def load_x(c):
        xt = xpool.tile([P, KT, CH], BF16, tag="xt", name="xt")
        nc.sync.dma_start(out=xt[:], in_=xT_v[:, :, c * CH:(c + 1) * CH])
        return xt

    pending = []
    post_pieces = []
    xts = {0: load_x(0)}
    for c in range(NCHUNK):
        if c + 1 < NCHUNK:
            xts[c + 1] = load_x(c + 1)
        xt = xts.pop(c)
        for p in "qkv":
            if c == NCHUNK - 1 and p == "v":
                pre, post_pieces = last_chunk_pieces(c)
                pending.extend(pre)
            asm = asmp.tile([P, CH // 8, G, 8], BF16, tag=f"asm{p}",
                            name=f"asm{p}")
            chunk_asm.setdefault(c, {})[p] = asm
            for m in range(MT):
                w = wpool.tile([P, KT, P], BF16, tag="wt", name="wt")
                nc.sync.dma_start(out=w[:], in_=wT_v[p][:, :, m * P:(m + 1) * P])
                ps = pp_ps.tile([P, CH], F32, tag="pp", name="pp")
                for k in range(KT):
                    nc.tensor.matmul(
                        ps[:],
                        lhsT=w[:, k, :],
                        rhs=xt[:, k, :],
                        start=(k == 0),
                        stop=(k == KT - 1),
                    )
                # bias + cast + scatter into the interleaved layout; alternate
                # ACT/DVE so neither engine's queue delays the attention chain
                dst = asm[:, :, m, :]
                src = ps[:].rearrange("p (b s) -> p b s", s=8)
                if m % 2 == 0:
                    nc.scalar.activation(out=dst, in_=src, func=AF.Identity,
                                         bias=bias_sb[p][:, m:m + 1], scale=1.0)
                else:
                    nc.vector.tensor_scalar_add(dst, src, bias_sb[p][:, m:m + 1])
                if pending:
                    pending.pop(0)()
                    if len(pending) > 16 and m % 2 == 0:
                        pending.pop(0)()
        if c < NCHUNK - 1:
            pending.extend(chunk_pieces(c))

    # drain the last chunk's attention (B pieces + epilogues)
    for piece in pending + post_pieces:
        piece()


_PROGRAM = None


def _build():
    global _PROGRAM
    if _PROGRAM is not None:
        return _PROGRAM
    from contextlib import ExitStack

    nc = bacc.Bacc("TRN2", target_bir_lowering=False, debug=False,
                   num_devices=N_CORES)
    with tile.TileContext(nc) as tc:
        with ExitStack() as ctx:
            _emit(nc, tc, ctx)
    nc.compile()
    _PROGRAM = nc
    return nc


def _host_inputs(x, Wq, bq, Wk, bk, Wv, bv):
    """Build the per-core input maps (host-side shard + transpose + cast)."""
    scale = 1.0 / np.sqrt(DG)
    xf = np.ascontiguousarray(x.reshape(-1, D))           # [16384, D]
    assert xf.shape[0] == N_CORES * TC

    bf = ml_dtypes.bfloat16
    shared = {
        "wqT": np.ascontiguousarray((Wq * scale).T).astype(bf),
        "wkT": np.ascontiguousarray(Wk.T).astype(bf),
        "wvT": np.ascontiguousarray(Wv.T).astype(bf),
        "bq": np.ascontiguousarray((bq * scale).reshape(G, DG).T).astype(np.float32),
        "bk": np.ascontiguousarray(bk.reshape(G, DG).T).astype(np.float32),
        "bv": np.ascontiguousarray(bv.reshape(G, DG).T).astype(np.float32),
        "m01": np.ascontiguousarray(np.broadcast_to(
            np.kron(np.ones((G, G), dtype=np.float32),
                    np.eye(8, dtype=np.float32))[:, None, :],
            (P, 4, P))),
        "ident": np.eye(P, dtype=np.float32).astype(bf),
    }
    in_maps = []
    for i in range(N_CORES):
        xi = xf[i * TC:(i + 1) * TC]
        m = dict(shared)
        m["xT"] = np.ascontiguousarray(xi.T).astype(bf)
        in_maps.append(m)
    return in_maps


last_results = None


def _install_ntff_shim():
    """Provide antenv.axon_hooks if the image lacks it (profiling only)."""
    import sys
    try:
        from antenv.axon_hooks import get_axon_ntff_profile_hook  # noqa: F401
        return
    except ImportError:
        pass
    import contextlib
    import ctypes
    import types

    so_path = "/opt/axon/libaxon_pjrt.so"
    hook = None
    if os.path.exists(so_path):
        lib = ctypes.CDLL(so_path)
        if hasattr(lib, "axon_start_nrt_profile"):
            lib.axon_start_nrt_profile.argtypes = [
                ctypes.POINTER(ctypes.c_int64), ctypes.c_size_t]
            lib.axon_start_nrt_profile.restype = ctypes.c_int64
            lib.axon_stop_nrt_profile.argtypes = [ctypes.c_char_p]
            lib.axon_stop_nrt_profile.restype = ctypes.c_int64

            @contextlib.contextmanager
            def _hook(output_dir, device_ids):
                import jax
                jax.devices()
                if device_ids:
                    ids = (ctypes.c_int64 * len(device_ids))(*device_ids)
                    rc = lib.axon_start_nrt_profile(ids, len(device_ids))
                else:
                    rc = lib.axon_start_nrt_profile(None, 0)
                if rc != 0:
                    raise RuntimeError(f"axon_start_nrt_profile rc={rc}")
                try:
                    yield
                finally:
                    n = lib.axon_stop_nrt_profile(str(output_dir).encode())
                    print(f"profile: {n} file(s) written to {output_dir}")

            hook = _hook

    mod = types.ModuleType("antenv.axon_hooks")
    mod.get_axon_ntff_profile_hook = lambda: hook
    mod.set_axon_ntff_profile_hook = lambda h: None
    import antenv
    antenv.axon_hooks = mod
    sys.modules["antenv.axon_hooks"] = mod


def kernel(**inputs):
    global last_results
    nc = _build()
    in_maps = _host_inputs(**inputs)
    trace = bool(os.environ.get("BASS_TRACE"))
    if trace:
        _install_ntff_shim()
    res = run_bass_kernel_spmd(nc, in_maps, list(range(N_CORES)), trace=trace)
    last_results = res
    x = inputs["x"]
    out = np.empty((N_CORES * TC, D), dtype=np.float32)
    for i in range(N_CORES):
        out[i * TC:(i + 1) * TC] = res.results[i]["outT"].T
    return out.reshape(x.shape)



# revision 4
# speedup vs baseline: 1.0493x; 1.0493x over previous
"""Trainium2 Bass kernel for per-token grouped attention (GQA-style).

Computation (per token t):
    q = x @ Wq.T + bq ; k = x @ Wk.T + bk ; v = x @ Wv.T + bv     (D=2048)
    reshape to (G=16 groups, d=128); scores = q_g . k_h / sqrt(d) (16x16)
    att = softmax(scores, axis=h); out = att @ v  -> (G*d,)

Sharding: data-parallel over the B*T = 16384 tokens across 8 cores
(2048 tokens/core).  Device works feature-major for the projections; the
attention emits the output token-major ([(g,s), block, dd]) and the host
unscrambles.

Device program (per core, SPMD), 4 chunks of 512 tokens:
  Projections: qT/kT/vT = W.T-tiles @ xT, bf16 matmuls with fp32 PSUM
    accumulation, bias added during the PSUM->SBUF copy (ACT), scattered
    into block-interleaved SBUF tiles [dd, block, g, s] (single-buffered).
  Attention per 8-token block b (3.01 matmul-equivalents instead of 4):
    sT = k_blk^T q_blk  (one 128x128 MM: all 64 pairwise 16x16 tiles,
         only the 8 diagonal ones survive the mask)
    e  = exp(sT) * blockdiag-mask          (ACT + DVE)
    vT = PE-transpose(v_blk)               (1 MM)
    o  = e^T @ vT   -> out^T[(g,s), dd]    (1 MM, unnormalized)
    dn = e^T @ ones -> softmax denominators (1-column MM, ~free)
    out = o * (1/dn) broadcast             (DVE, batched per super-block)
  Pump schedule (keeps PE fed, allows single-buffered q/k/v tiles):
    chunk c's q,k slots <- att@v pieces of chunk c-1
    chunk c's v slots   <- scores pieces of chunk c
    after last chunk    <- drain att@v of last chunk
"""

import os
import numpy as np
import ml_dtypes

import concourse.bass as bass
import concourse.tile as tile
from concourse import bacc, mybir
from concourse.bass_utils import run_bass_kernel_spmd

F32 = mybir.dt.float32
BF16 = mybir.dt.bfloat16
AF = mybir.ActivationFunctionType
ALU = mybir.AluOpType

P = 128          # SBUF partitions
D = 2048         # model dim
G = 16           # groups
DG = 128         # per-group dim
N_CORES = 8
TC = 2048        # tokens per core
NCHUNK = 4
CH = TC // NCHUNK          # 512 tokens per chunk
NB = CH // 8               # 64 blocks of 8 tokens per chunk
NSB = NB // 4              # 16 super-blocks (32 tokens) per chunk
KT = D // P      # 16 contraction tiles
MT = D // P      # 16 output-feature tiles
OTB = 32         # blocks per output tile (256 tokens)


def _emit(nc, tc, ctx):
    # ---- DRAM I/O -------------------------------------------------------
    xT = nc.dram_tensor("xT", [D, TC], BF16, kind="ExternalInput").ap()
    wT = {
        p: nc.dram_tensor(f"w{p}T", [D, D], BF16, kind="ExternalInput").ap()
        for p in "qkv"
    }
    b_dram = {
        p: nc.dram_tensor(f"b{p}", [P, G], F32, kind="ExternalInput").ap()
        for p in "qkv"
    }
    m01_dram = nc.dram_tensor("m01", [P, P], BF16, kind="ExternalInput").ap()
    ident_dram = nc.dram_tensor("ident", [P, P], BF16, kind="ExternalInput").ap()
    ones_dram = nc.dram_tensor("ones", [P, 1], BF16, kind="ExternalInput").ap()
    outT = nc.dram_tensor("outT", [P, TC // 8, P], BF16,
                          kind="ExternalOutput").ap()

    # ---- pools ----------------------------------------------------------
    singles = ctx.enter_context(tc.tile_pool(name="singles", bufs=1))
    xpool = ctx.enter_context(tc.tile_pool(name="xpool", bufs=2))
    wpool = ctx.enter_context(tc.tile_pool(name="wpool", bufs=4))
    asmp = ctx.enter_context(tc.tile_pool(name="asmp", bufs=1))
    epool = ctx.enter_context(tc.tile_pool(name="epool", bufs=NSB + 2))
    vtpool = ctx.enter_context(tc.tile_pool(name="vtpool", bufs=3))
    recp = ctx.enter_context(tc.tile_pool(name="recp", bufs=3))
    otp = ctx.enter_context(tc.tile_pool(name="otp", bufs=2))

    pp_ps = ctx.enter_context(tc.tile_pool(name="pp_ps", bufs=2, space="PSUM"))
    ps_s = ctx.enter_context(tc.tile_pool(name="ps_s", bufs=2, space="PSUM"))
    ps_vt = ctx.enter_context(tc.tile_pool(name="ps_vt", bufs=1, space="PSUM"))
    ps_o = ctx.enter_context(tc.tile_pool(name="ps_o", bufs=2, space="PSUM"))
    ps_dn = ctx.enter_context(tc.tile_pool(name="ps_dn", bufs=1, space="PSUM"))

    # ---- constants ------------------------------------------------------
    m01_sb = singles.tile([P, P], BF16, tag="m01", name="m01")
    nc.sync.dma_start(out=m01_sb[:], in_=m01_dram[:])
    ident_sb = singles.tile([P, P], BF16, tag="ident", name="ident")
    nc.sync.dma_start(out=ident_sb[:], in_=ident_dram[:])
    ones_sb = singles.tile([P, 1], BF16, tag="ones", name="ones")
    nc.sync.dma_start(out=ones_sb[:], in_=ones_dram[:])
    bias_sb = {}
    for p in "qkv":
        bias_sb[p] = singles.tile([P, G], F32, tag=f"bias{p}", name=f"bias{p}")
        nc.sync.dma_start(out=bias_sb[p][:], in_=b_dram[p][:])

    # DRAM views
    xT_v = xT.rearrange("(k p) t -> p k t", p=P)          # [P, KT, TC]
    wT_v = {p: wT[p].rearrange("(k p) o -> p k o", p=P) for p in "qkv"}

    # per-chunk assembled q/k/v (block-interleaved [dd, block, g, s]),
    # single-buffered: the pump schedule guarantees producer/consumer order.
    asm = {}

    # ---- attention pieces ----------------------------------------------
    st_e = {}       # (sb) -> masked exp tile for current chunk's scores
    st_ot = {}      # out tile in progress

    def piece_a(c, sb):
        """Scores + exp + mask for super-block sb of chunk c."""
        q2f = asm["q"].rearrange("p b g s -> p (b g s)")
        k2f = asm["k"].rearrange("p b g s -> p (b g s)")
        sT = ps_s.tile([P, 4, P], F32, tag="s", name="s")
        for j in range(4):
            sl = slice((sb * 4 + j) * P, (sb * 4 + j + 1) * P)
            nc.tensor.matmul(sT[:, j, :], lhsT=k2f[:, sl], rhs=q2f[:, sl],
                             start=True, stop=True)
        e = epool.tile([P, 4, P], BF16, tag="e", name="e")
        nc.scalar.activation(out=e[:], in_=sT[:], func=AF.Exp)
        m01_bc = m01_sb[:].unsqueeze(1).broadcast_to([P, 4, P])
        nc.vector.tensor_tensor(out=e[:], in0=e[:], in1=m01_bc, op=ALU.mult)
        st_e[sb] = e

    def piece_b(c, sb):
        """v-transpose + att@v + denominators + normalize for sb of chunk c."""
        if sb % 8 == 0:
            st_ot["t"] = otp.tile([P, OTB, P], BF16, tag="ot", name="ot")
        e = st_e.pop(sb)
        v2f = asm["v"].rearrange("p b g s -> p (b g s)")
        vt_ps = ps_vt.tile([P, 4, P], BF16, tag="vt", name="vt")
        for j in range(4):
            sl = slice((sb * 4 + j) * P, (sb * 4 + j + 1) * P)
            nc.tensor.transpose(vt_ps[:, j, :], v2f[:, sl], ident_sb[:])
        vt = vtpool.tile([P, 4, P], BF16, tag="vts", name="vts")
        nc.vector.tensor_copy(out=vt[:], in_=vt_ps[:])
        o_ps = ps_o.tile([P, 4, P], F32, tag="o", name="o")
        dn_ps = ps_dn.tile([P, 4], F32, tag="dn", name="dn")
        for j in range(4):
            nc.tensor.matmul(o_ps[:, j, :], lhsT=e[:, j, :], rhs=vt[:, j, :],
                             start=True, stop=True)
            nc.tensor.matmul(dn_ps[:, j:j + 1], lhsT=e[:, j, :],
                             rhs=ones_sb[:], start=True, stop=True)
        rec = recp.tile([P, 4], F32, tag="rec", name="rec")
        nc.vector.reciprocal(out=rec[:], in_=dn_ps[:])
        rec_bc = rec[:].unsqueeze(2).broadcast_to([P, 4, P])
        dst = st_ot["t"][:, (sb % 8) * 4:(sb % 8) * 4 + 4, :]
        nc.vector.tensor_tensor(out=dst, in0=o_ps[:], in1=rec_bc, op=ALU.mult)
        if sb % 8 == 7:
            t0 = c * NB + (sb - 7) * 4          # first block of this out tile
            nc.gpsimd.dma_start(out=outT[:, t0:t0 + OTB, :], in_=st_ot["t"][:])

    # ---- projections with attention pieces pumped in --------------------
    def load_x(c):
        xt = xpool.tile([P, KT, CH], BF16, tag="xt", name="xt")
        nc.sync.dma_start(out=xt[:], in_=xT_v[:, :, c * CH:(c + 1) * CH])
        return xt

    def proj_slot(p, m, xt):
        w = wpool.tile([P, KT, P], BF16, tag="wt", name="wt")
        nc.sync.dma_start(out=w[:], in_=wT_v[p][:, :, m * P:(m + 1) * P])
        ps = pp_ps.tile([P, CH], F32, tag="pp", name="pp")
        for k in range(KT):
            nc.tensor.matmul(ps[:], lhsT=w[:, k, :], rhs=xt[:, k, :],
                             start=(k == 0), stop=(k == KT - 1))
        dst = asm[p][:, :, m, :]
        src = ps[:].rearrange("p (b s) -> p b s", s=8)
        nc.scalar.activation(out=dst, in_=src, func=AF.Identity,
                             bias=bias_sb[p][:, m:m + 1], scale=1.0)

    xts = {0: load_x(0)}
    for c in range(NCHUNK):
        if c + 1 < NCHUNK:
            xts[c + 1] = load_x(c + 1)
        xt = xts.pop(c)
        pending_b = [lambda sb=sb: piece_b(c - 1, sb) for sb in range(NSB)] \
            if c > 0 else []
        # q,k slots: pump previous chunk's att@v pieces (1 per 2 slots)
        for i, p in enumerate("qk"):
            asm[p] = asmp.tile([P, NB, G, 8], BF16, tag=f"asm{p}",
                               name=f"asm{p}")
            for m in range(MT):
                proj_slot(p, m, xt)
                if (i * MT + m) % 2 == 1 and pending_b:
                    pending_b.pop(0)()
        while pending_b:
            pending_b.pop(0)()
        # v slots: pump this chunk's scores pieces (1 per slot)
        asm["v"] = asmp.tile([P, NB, G, 8], BF16, tag="asmv", name="asmv")
        for m in range(MT):
            proj_slot("v", m, xt)
            if m < NSB:
                piece_a(c, m)
    # drain: att@v of the last chunk
    for sb in range(NSB):
        piece_b(NCHUNK - 1, sb)


_PROGRAM = None


def _build():
    global _PROGRAM
    if _PROGRAM is not None:
        return _PROGRAM
    from contextlib import ExitStack

    nc = bacc.Bacc("TRN2", target_bir_lowering=False, debug=False,
                   num_devices=N_CORES)
    with tile.TileContext(nc) as tc:
        with ExitStack() as ctx:
            _emit(nc, tc, ctx)
    nc.compile()
    _PROGRAM = nc
    return nc


def _host_inputs(x, Wq, bq, Wk, bk, Wv, bv):
    """Build the per-core input maps (host-side shard + transpose + cast)."""
    scale = 1.0 / np.sqrt(DG)
    xf = np.ascontiguousarray(x.reshape(-1, D))           # [16384, D]
    assert xf.shape[0] == N_CORES * TC

    bf = ml_dtypes.bfloat16
    shared = {
        "wqT": np.ascontiguousarray((Wq * scale).T).astype(bf),
        "wkT": np.ascontiguousarray(Wk.T).astype(bf),
        "wvT": np.ascontiguousarray(Wv.T).astype(bf),
        "bq": np.ascontiguousarray((bq * scale).reshape(G, DG).T).astype(np.float32),
        "bk": np.ascontiguousarray(bk.reshape(G, DG).T).astype(np.float32),
        "bv": np.ascontiguousarray(bv.reshape(G, DG).T).astype(np.float32),
        "m01": np.ascontiguousarray(
            np.kron(np.ones((G, G), dtype=np.float32),
                    np.eye(8, dtype=np.float32))).astype(bf),
        "ident": np.eye(P, dtype=np.float32).astype(bf),
        "ones": np.ones((P, 1), dtype=np.float32).astype(bf),
    }
    in_maps = []
    for i in range(N_CORES):
        xi = xf[i * TC:(i + 1) * TC]
        m = dict(shared)
        m["xT"] = np.ascontiguousarray(xi.T).astype(bf)
        in_maps.append(m)
    return in_maps


last_results = None


def _install_ntff_shim():
    """Provide antenv.axon_hooks if the image lacks it (profiling only)."""
    import sys
    try:
        from antenv.axon_hooks import get_axon_ntff_profile_hook  # noqa: F401
        return
    except ImportError:
        pass
    import contextlib
    import ctypes
    import types

    so_path = "/opt/axon/libaxon_pjrt.so"
    hook = None
    if os.path.exists(so_path):
        lib = ctypes.CDLL(so_path)
        if hasattr(lib, "axon_start_nrt_profile"):
            lib.axon_start_nrt_profile.argtypes = [
                ctypes.POINTER(ctypes.c_int64), ctypes.c_size_t]
            lib.axon_start_nrt_profile.restype = ctypes.c_int64
            lib.axon_stop_nrt_profile.argtypes = [ctypes.c_char_p]
            lib.axon_stop_nrt_profile.restype = ctypes.c_int64

            @contextlib.contextmanager
            def _hook(output_dir, device_ids):
                import jax
                jax.devices()
                if device_ids:
                    ids = (ctypes.c_int64 * len(device_ids))(*device_ids)
                    rc = lib.axon_start_nrt_profile(ids, len(device_ids))
                else:
                    rc = lib.axon_start_nrt_profile(None, 0)
                if rc != 0:
                    raise RuntimeError(f"axon_start_nrt_profile rc={rc}")
                try:
                    yield
                finally:
                    n = lib.axon_stop_nrt_profile(str(output_dir).encode())
                    print(f"profile: {n} file(s) written to {output_dir}")

            hook = _hook

    mod = types.ModuleType("antenv.axon_hooks")
    mod.get_axon_ntff_profile_hook = lambda: hook
    mod.set_axon_ntff_profile_hook = lambda h: None
    import antenv
    antenv.axon_hooks = mod
    sys.modules["antenv.axon_hooks"] = mod


def kernel(**inputs):
    global last_results
    nc = _build()
    in_maps = _host_inputs(**inputs)
    trace = bool(os.environ.get("BASS_TRACE"))
    if trace:
        _install_ntff_shim()
    res = run_bass_kernel_spmd(nc, in_maps, list(range(N_CORES)), trace=trace)
    last_results = res
    x = inputs["x"]
    out = np.empty((N_CORES * TC, D), dtype=np.float32)
    for i in range(N_CORES):
        o = res.results[i]["outT"].astype(np.float32)      # [P, TC/8, P]
        o = o.reshape(G, 8, TC // 8, DG).transpose(2, 1, 0, 3)
        out[i * TC:(i + 1) * TC] = o.reshape(TC, D)
    return out.reshape(x.shape)


# revision 7
# speedup vs baseline: 1.0546x; 1.0051x over previous
"""Trainium2 Bass kernel for per-token grouped attention (GQA-style).

Computation (per token t):
    q = x @ Wq.T + bq ; k = x @ Wk.T + bk ; v = x @ Wv.T + bv     (D=2048)
    reshape to (G=16 groups, d=128); scores = q_g . k_h / sqrt(d) (16x16)
    att = softmax(scores, axis=h); out = att @ v  -> (G*d,)

Sharding: data-parallel over the B*T = 16384 tokens across 8 cores
(2048 tokens/core).  Device works feature-major for the projections; the
attention emits the output token-major ([(g,s), block, dd]) and the host
unscrambles.

Device program (per core, SPMD), 4 chunks of 512 tokens:
  Projections: qT/kT/vT = W.T-tiles @ xT, bf16 matmuls with fp32 PSUM
    accumulation, bias added during the PSUM->SBUF copy (ACT), scattered
    into block-interleaved SBUF tiles [dd, block, g, s] (single-buffered).
  Attention per 8-token block b (3.01 matmul-equivalents instead of 4):
    sT = k_blk^T q_blk  (one 128x128 MM: all 64 pairwise 16x16 tiles,
         only the 8 diagonal ones survive the mask)
    e  = exp(sT) * blockdiag-mask          (ACT + DVE)
    vT = PE-transpose(v_blk)               (1 MM)
    o  = e^T @ vT   -> out^T[(g,s), dd]    (1 MM, unnormalized)
    dn = e^T @ ones -> softmax denominators (1-column MM, ~free)
    out = o * (1/dn) broadcast             (DVE, batched per super-block)
  Pump schedule (keeps PE fed, allows single-buffered q/k/v tiles):
    chunk c's q,k slots <- att@v pieces of chunk c-1
    chunk c's v slots   <- scores pieces of chunk c
    after last chunk    <- drain att@v of last chunk
"""

import os
import numpy as np
import ml_dtypes

import concourse.bass as bass
import concourse.tile as tile
from concourse import bacc, mybir
from concourse.bass_utils import run_bass_kernel_spmd

F32 = mybir.dt.float32
BF16 = mybir.dt.bfloat16
AF = mybir.ActivationFunctionType
ALU = mybir.AluOpType

P = 128          # SBUF partitions
D = 2048         # model dim
G = 16           # groups
DG = 128         # per-group dim
N_CORES = 8
TC = 2048        # tokens per core
NCHUNK = 4
CH = TC // NCHUNK          # 512 tokens per chunk
NB = CH // 8               # 64 blocks of 8 tokens per chunk
NSB = NB // 4              # 16 super-blocks (32 tokens) per chunk
KT = D // P      # 16 contraction tiles
MT = D // P      # 16 output-feature tiles
OTB = 32         # blocks per output tile (256 tokens)


def _emit(nc, tc, ctx):
    # ---- DRAM I/O -------------------------------------------------------
    xT = nc.dram_tensor("xT", [D, TC], BF16, kind="ExternalInput").ap()
    wT = {
        p: nc.dram_tensor(f"w{p}T", [D, D], BF16, kind="ExternalInput").ap()
        for p in "qkv"
    }
    b_dram = {
        p: nc.dram_tensor(f"b{p}", [P, G], F32, kind="ExternalInput").ap()
        for p in "qkv"
    }
    m01_dram = nc.dram_tensor("m01", [P, P], BF16, kind="ExternalInput").ap()
    ident_dram = nc.dram_tensor("ident", [P, P], BF16, kind="ExternalInput").ap()
    outT = nc.dram_tensor("outT", [P, TC // 8, P], BF16,
                          kind="ExternalOutput").ap()

    # ---- pools ----------------------------------------------------------
    singles = ctx.enter_context(tc.tile_pool(name="singles", bufs=1))
    xpool = ctx.enter_context(tc.tile_pool(name="xpool", bufs=2))
    wpool = ctx.enter_context(tc.tile_pool(name="wpool", bufs=6))
    asmp = ctx.enter_context(tc.tile_pool(name="asmp", bufs=1))
    epool = ctx.enter_context(tc.tile_pool(name="epool", bufs=NSB + 2))
    vtpool = ctx.enter_context(tc.tile_pool(name="vtpool", bufs=3))
    recp = ctx.enter_context(tc.tile_pool(name="recp", bufs=3))
    otp = ctx.enter_context(tc.tile_pool(name="otp", bufs=2))

    pp_ps = ctx.enter_context(tc.tile_pool(name="pp_ps", bufs=2, space="PSUM"))
    ps_s = ctx.enter_context(tc.tile_pool(name="ps_s", bufs=2, space="PSUM"))
    ps_vt = ctx.enter_context(tc.tile_pool(name="ps_vt", bufs=1, space="PSUM"))
    ps_o = ctx.enter_context(tc.tile_pool(name="ps_o", bufs=3, space="PSUM"))

    # ---- constants ------------------------------------------------------
    m01_sb = singles.tile([P, P], BF16, tag="m01", name="m01")
    nc.sync.dma_start(out=m01_sb[:], in_=m01_dram[:])
    ident_sb = singles.tile([P, P], BF16, tag="ident", name="ident")
    nc.sync.dma_start(out=ident_sb[:], in_=ident_dram[:])
    bias_sb = {}
    for p in "qkv":
        bias_sb[p] = singles.tile([P, G], F32, tag=f"bias{p}", name=f"bias{p}")
        nc.sync.dma_start(out=bias_sb[p][:], in_=b_dram[p][:])

    # DRAM views
    xT_v = xT.rearrange("(k p) t -> p k t", p=P)          # [P, KT, TC]
    wT_v = {p: wT[p].rearrange("(k p) o -> p k o", p=P) for p in "qkv"}

    # pre-warm the vt ring: the 129th column stays 1.0 forever (the
    # ones-feature that makes att@v emit softmax denominators in column P)
    for _ in range(3):
        vtw = vtpool.tile([P, 4, P + 1], BF16, tag="vts", name="vts")
        nc.vector.memset(vtw[:, :, P:P + 1], 1.0)

    # per-chunk assembled q/k/v (block-interleaved [dd, block, g, s]),
    # single-buffered: the pump schedule guarantees producer/consumer order.
    asm = {}

    # ---- attention pieces ----------------------------------------------
    st_e = {}       # (sb) -> masked exp tile for current chunk's scores
    st_ot = {}      # out tile in progress

    def piece_a(c, sb):
        """Scores + exp + mask for super-block sb of chunk c."""
        q2f = asm["q"].rearrange("p b g s -> p (b g s)")
        k2f = asm["k"].rearrange("p b g s -> p (b g s)")
        sT = ps_s.tile([P, 4, P], F32, tag="s", name="s")
        for j in range(4):
            sl = slice((sb * 4 + j) * P, (sb * 4 + j + 1) * P)
            nc.tensor.matmul(sT[:, j, :], lhsT=k2f[:, sl], rhs=q2f[:, sl],
                             start=True, stop=True)
        e = epool.tile([P, 4, P], BF16, tag="e", name="e")
        nc.scalar.activation(out=e[:], in_=sT[:], func=AF.Exp)
        m01_bc = m01_sb[:].unsqueeze(1).broadcast_to([P, 4, P])
        nc.vector.tensor_tensor(out=e[:], in0=e[:], in1=m01_bc, op=ALU.mult)
        st_e[sb] = e

    def piece_b(c, sb):
        """v-transpose + att@v (with fused denominators) + normalize."""
        if sb % 8 == 0:
            st_ot["t"] = otp.tile([P, OTB, P], BF16, tag="ot", name="ot")
        e = st_e.pop(sb)
        v2f = asm["v"].rearrange("p b g s -> p (b g s)")
        vt_ps = ps_vt.tile([P, 4, P], BF16, tag="vt", name="vt")
        for j in range(4):
            sl = slice((sb * 4 + j) * P, (sb * 4 + j + 1) * P)
            nc.tensor.transpose(vt_ps[:, j, :], v2f[:, sl], ident_sb[:])
        # vt has a 129th column preset to 1.0 (ones-feature -> denominators)
        vt = vtpool.tile([P, 4, P + 1], BF16, tag="vts", name="vts")
        nc.scalar.copy(out=vt[:, :, 0:P], in_=vt_ps[:])
        for h in range(2):
            o_ps = ps_o.tile([P, 2, P + 1], F32, tag="o", name="o")
            for jj in range(2):
                j = h * 2 + jj
                nc.tensor.matmul(o_ps[:, jj, :], lhsT=e[:, j, :],
                                 rhs=vt[:, j, :], start=True, stop=True)
            rec = recp.tile([P, 2, 1], F32, tag="rec", name="rec")
            nc.vector.reciprocal(out=rec[:], in_=o_ps[:, :, P:P + 1])
            rec_bc = rec[:].broadcast_to([P, 2, P])
            dst = st_ot["t"][:, (sb % 8) * 4 + h * 2:(sb % 8) * 4 + h * 2 + 2, :]
            nc.vector.tensor_tensor(out=dst, in0=o_ps[:, :, 0:P],
                                    in1=rec_bc, op=ALU.mult)
        if sb % 8 == 7:
            t0 = c * NB + (sb - 7) * 4          # first block of this out tile
            nc.gpsimd.dma_start(out=outT[:, t0:t0 + OTB, :], in_=st_ot["t"][:])

    # ---- projections with attention pieces pumped in --------------------
    def load_x(c):
        xt = xpool.tile([P, KT, CH], BF16, tag="xt", name="xt")
        nc.gpsimd.dma_start(out=xt[:], in_=xT_v[:, :, c * CH:(c + 1) * CH])
        return xt

    def proj_slot(p, m, xt):
        w = wpool.tile([P, KT, P], BF16, tag="wt", name="wt")
        nc.sync.dma_start(out=w[:], in_=wT_v[p][:, :, m * P:(m + 1) * P])
        ps = pp_ps.tile([P, CH], F32, tag="pp", name="pp")
        for k in range(KT):
            nc.tensor.matmul(ps[:], lhsT=w[:, k, :], rhs=xt[:, k, :],
                             start=(k == 0), stop=(k == KT - 1))
        dst = asm[p][:, :, m, :]
        src = ps[:].rearrange("p (b s) -> p b s", s=8)
        nc.scalar.activation(out=dst, in_=src, func=AF.Identity,
                             bias=bias_sb[p][:, m:m + 1], scale=1.0)

    xts = {0: load_x(0)}
    for c in range(NCHUNK):
        xt = xts.pop(c)
        pending_b = [lambda sb=sb: piece_b(c - 1, sb) for sb in range(NSB)] \
            if c > 0 else []
        # q,k slots: pump previous chunk's att@v pieces (1 per 2 slots)
        for i, p in enumerate("qk"):
            asm[p] = asmp.tile([P, NB, G, 8], BF16, tag=f"asm{p}",
                               name=f"asm{p}")
            for m in range(MT):
                proj_slot(p, m, xt)
                if (i * MT + m) % 2 == 1 and pending_b:
                    pending_b.pop(0)()
        while pending_b:
            pending_b.pop(0)()
        # v slots: pump this chunk's scores pieces (1 per slot)
        if c + 1 < NCHUNK:
            xts[c + 1] = load_x(c + 1)
        asm["v"] = asmp.tile([P, NB, G, 8], BF16, tag="asmv", name="asmv")
        for m in range(MT):
            proj_slot("v", m, xt)
            if m < NSB:
                piece_a(c, m)
    # drain: att@v of the last chunk
    for sb in range(NSB):
        piece_b(NCHUNK - 1, sb)


_PROGRAM = None


def _build():
    global _PROGRAM
    if _PROGRAM is not None:
        return _PROGRAM
    from contextlib import ExitStack

    nc = bacc.Bacc("TRN2", target_bir_lowering=False, debug=False,
                   num_devices=N_CORES)
    with tile.TileContext(nc) as tc:
        with ExitStack() as ctx:
            _emit(nc, tc, ctx)
    nc.compile()
    _PROGRAM = nc
    return nc


def _host_inputs(x, Wq, bq, Wk, bk, Wv, bv):
    """Build the per-core input maps (host-side shard + transpose + cast)."""
    scale = 1.0 / np.sqrt(DG)
    xf = np.ascontiguousarray(x.reshape(-1, D))           # [16384, D]
    assert xf.shape[0] == N_CORES * TC

    bf = ml_dtypes.bfloat16
    shared = {
        "wqT": np.ascontiguousarray((Wq * scale).T).astype(bf),
        "wkT": np.ascontiguousarray(Wk.T).astype(bf),
        "wvT": np.ascontiguousarray(Wv.T).astype(bf),
        "bq": np.ascontiguousarray((bq * scale).reshape(G, DG).T).astype(np.float32),
        "bk": np.ascontiguousarray(bk.reshape(G, DG).T).astype(np.float32),
        "bv": np.ascontiguousarray(bv.reshape(G, DG).T).astype(np.float32),
        "m01": np.ascontiguousarray(
            np.kron(np.ones((G, G), dtype=np.float32),
                    np.eye(8, dtype=np.float32))).astype(bf),
        "ident": np.eye(P, dtype=np.float32).astype(bf),
    }
    in_maps = []
    for i in range(N_CORES):
        xi = xf[i * TC:(i + 1) * TC]
        m = dict(shared)
        m["xT"] = np.ascontiguousarray(xi.T).astype(bf)
        in_maps.append(m)
    return in_maps


last_results = None


def _install_ntff_shim():
    """Provide antenv.axon_hooks if the image lacks it (profiling only)."""
    import sys
    try:
        from antenv.axon_hooks import get_axon_ntff_profile_hook  # noqa: F401
        return
    except ImportError:
        pass
    import contextlib
    import ctypes
    import types

    so_path = "/opt/axon/libaxon_pjrt.so"
    hook = None
    if os.path.exists(so_path):
        lib = ctypes.CDLL(so_path)
        if hasattr(lib, "axon_start_nrt_profile"):
            lib.axon_start_nrt_profile.argtypes = [
                ctypes.POINTER(ctypes.c_int64), ctypes.c_size_t]
            lib.axon_start_nrt_profile.restype = ctypes.c_int64
            lib.axon_stop_nrt_profile.argtypes = [ctypes.c_char_p]
            lib.axon_stop_nrt_profile.restype = ctypes.c_int64

            @contextlib.contextmanager
            def _hook(output_dir, device_ids):
                import jax
                jax.devices()
                if device_ids:
                    ids = (ctypes.c_int64 * len(device_ids))(*device_ids)
                    rc = lib.axon_start_nrt_profile(ids, len(device_ids))
                else:
                    rc = lib.axon_start_nrt_profile(None, 0)
                if rc != 0:
                    raise RuntimeError(f"axon_start_nrt_profile rc={rc}")
                try:
                    yield
                finally:
                    n = lib.axon_stop_nrt_profile(str(output_dir).encode())
                    print(f"profile: {n} file(s) written to {output_dir}")

            hook = _hook

    mod = types.ModuleType("antenv.axon_hooks")
    mod.get_axon_ntff_profile_hook = lambda: hook
    mod.set_axon_ntff_profile_hook = lambda h: None
    import antenv
    antenv.axon_hooks = mod
    sys.modules["antenv.axon_hooks"] = mod


def kernel(**inputs):
    global last_results
    nc = _build()
    in_maps = _host_inputs(**inputs)
    trace = bool(os.environ.get("BASS_TRACE"))
    if trace:
        _install_ntff_shim()
    res = run_bass_kernel_spmd(nc, in_maps, list(range(N_CORES)), trace=trace)
    last_results = res
    x = inputs["x"]
    out = np.empty((N_CORES * TC, D), dtype=np.float32)
    for i in range(N_CORES):
        o = res.results[i]["outT"].astype(np.float32)      # [P, TC/8, P]
        o = o.reshape(G, 8, TC // 8, DG).transpose(2, 1, 0, 3)
        out[i * TC:(i + 1) * TC] = o.reshape(TC, D)
    return out.reshape(x.shape)


# revision 8
# speedup vs baseline: 1.0698x; 1.0144x over previous
"""Trainium2 Bass kernel for per-token grouped attention (GQA-style).

Computation (per token t):
    q = x @ Wq.T + bq ; k = x @ Wk.T + bk ; v = x @ Wv.T + bv     (D=2048)
    reshape to (G=16 groups, d=128); scores = q_g . k_h / sqrt(d) (16x16)
    att = softmax(scores, axis=h); out = att @ v  -> (G*d,)

Sharding: data-parallel over the B*T = 16384 tokens across 8 cores
(2048 tokens/core).  Device works feature-major for the projections; the
attention emits the output token-major ([(g,s), block, dd]) and the host
unscrambles.

Device program (per core, SPMD), 4 chunks of 512 tokens:
  Projections: qT/kT/vT = W.T-tiles @ xT, bf16 matmuls with fp32 PSUM
    accumulation, bias added during the PSUM->SBUF copy (ACT), scattered
    into block-interleaved SBUF tiles [dd, block, g, s] (single-buffered).
  Attention per 8-token block b (3.01 matmul-equivalents instead of 4):
    sT = k_blk^T q_blk  (one 128x128 MM: all 64 pairwise 16x16 tiles,
         only the 8 diagonal ones survive the mask)
    e  = exp(sT) * blockdiag-mask          (ACT + DVE)
    vT = PE-transpose(v_blk)               (1 MM)
    o  = e^T @ vT   -> out^T[(g,s), dd]    (1 MM, unnormalized)
    dn = e^T @ ones -> softmax denominators (1-column MM, ~free)
    out = o * (1/dn) broadcast             (DVE, batched per super-block)
  Pump schedule (keeps PE fed, allows single-buffered q/k/v tiles):
    chunk c's q,k slots <- att@v pieces of chunk c-1
    chunk c's v slots   <- scores pieces of chunk c
    after last chunk    <- drain att@v of last chunk
"""

import os
import numpy as np
import ml_dtypes

import concourse.bass as bass
import concourse.tile as tile
from concourse import bacc, mybir
from concourse.bass_utils import run_bass_kernel_spmd

F32 = mybir.dt.float32
BF16 = mybir.dt.bfloat16
AF = mybir.ActivationFunctionType
ALU = mybir.AluOpType

P = 128          # SBUF partitions
D = 2048         # model dim
G = 16           # groups
DG = 128         # per-group dim
N_CORES = 8
TC = 2048        # tokens per core
NCHUNK = 4
CH = TC // NCHUNK          # 512 tokens per chunk
NB = CH // 8               # 64 blocks of 8 tokens per chunk
NSB = NB // 4              # 16 super-blocks (32 tokens) per chunk
KT = D // P      # 16 contraction tiles
MT = D // P      # 16 output-feature tiles
OTB = 32         # blocks per output tile (256 tokens)


def _emit(nc, tc, ctx):
    # ---- DRAM I/O -------------------------------------------------------
    xT = nc.dram_tensor("xT", [NCHUNK, P, KT, CH], BF16,
                        kind="ExternalInput").ap()
    wT = {
        p: nc.dram_tensor(f"w{p}T", [MT, P, KT, P], BF16,
                          kind="ExternalInput").ap()
        for p in "qkv"
    }
    b_dram = {
        p: nc.dram_tensor(f"b{p}", [P, G], F32, kind="ExternalInput").ap()
        for p in "qkv"
    }
    m01_dram = nc.dram_tensor("m01", [P, P], BF16, kind="ExternalInput").ap()
    ident_dram = nc.dram_tensor("ident", [P, P], BF16, kind="ExternalInput").ap()
    outT = nc.dram_tensor("outT", [P, TC // 8, P], BF16,
                          kind="ExternalOutput").ap()

    # ---- pools ----------------------------------------------------------
    singles = ctx.enter_context(tc.tile_pool(name="singles", bufs=1))
    xpool = ctx.enter_context(tc.tile_pool(name="xpool", bufs=2))
    wpool = ctx.enter_context(tc.tile_pool(name="wpool", bufs=6))
    asmp = ctx.enter_context(tc.tile_pool(name="asmp", bufs=1))
    epool = ctx.enter_context(tc.tile_pool(name="epool", bufs=NSB + 2))
    vtpool = ctx.enter_context(tc.tile_pool(name="vtpool", bufs=3))
    recp = ctx.enter_context(tc.tile_pool(name="recp", bufs=3))
    otp = ctx.enter_context(tc.tile_pool(name="otp", bufs=2))

    pp_ps = ctx.enter_context(tc.tile_pool(name="pp_ps", bufs=2, space="PSUM"))
    ps_s = ctx.enter_context(tc.tile_pool(name="ps_s", bufs=1, space="PSUM"))
    ps_vt = ctx.enter_context(tc.tile_pool(name="ps_vt", bufs=2, space="PSUM"))
    ps_o = ctx.enter_context(tc.tile_pool(name="ps_o", bufs=3, space="PSUM"))

    # ---- constants ------------------------------------------------------
    m01_sb = singles.tile([P, P], BF16, tag="m01", name="m01")
    nc.sync.dma_start(out=m01_sb[:], in_=m01_dram[:])
    ident_sb = singles.tile([P, P], BF16, tag="ident", name="ident")
    nc.sync.dma_start(out=ident_sb[:], in_=ident_dram[:])
    bias_sb = {}
    for p in "qkv":
        bias_sb[p] = singles.tile([P, G], F32, tag=f"bias{p}", name=f"bias{p}")
        nc.sync.dma_start(out=bias_sb[p][:], in_=b_dram[p][:])

    # pre-warm the vt ring: the 129th column stays 1.0 forever (the
    # ones-feature that makes att@v emit softmax denominators in column P)
    for _ in range(3):
        vtw = vtpool.tile([P, 4, P + 1], BF16, tag="vts", name="vts")
        nc.vector.memset(vtw[:, :, P:P + 1], 1.0)

    # per-chunk assembled q/k/v (block-interleaved [dd, block, g, s]),
    # single-buffered: the pump schedule guarantees producer/consumer order.
    asm = {}

    # ---- attention pieces ----------------------------------------------
    st_e = {}       # (sb) -> masked exp tile for current chunk's scores
    st_ot = {}      # out tile in progress

    def piece_a(c, sb):
        """Scores + exp + mask for super-block sb of chunk c."""
        q2f = asm["q"].rearrange("p b g s -> p (b g s)")
        k2f = asm["k"].rearrange("p b g s -> p (b g s)")
        sT = ps_s.tile([P, 4, P], F32, tag="s", name="s")
        for j in range(4):
            sl = slice((sb * 4 + j) * P, (sb * 4 + j + 1) * P)
            nc.tensor.matmul(sT[:, j, :], lhsT=k2f[:, sl], rhs=q2f[:, sl],
                             start=True, stop=True)
        e = epool.tile([P, 4, P], BF16, tag="e", name="e")
        nc.scalar.activation(out=e[:], in_=sT[:], func=AF.Exp)
        m01_bc = m01_sb[:].unsqueeze(1).broadcast_to([P, 4, P])
        nc.vector.tensor_tensor(out=e[:], in0=e[:], in1=m01_bc, op=ALU.mult)
        st_e[sb] = e

    def piece_b(c, sb):
        """v-transpose + att@v (with fused denominators) + normalize."""
        if sb % 8 == 0:
            st_ot["t"] = otp.tile([P, OTB, P], BF16, tag="ot", name="ot")
        e = st_e.pop(sb)
        v2f = asm["v"].rearrange("p b g s -> p (b g s)")
        vt_ps = ps_vt.tile([P, 4, P], BF16, tag="vt", name="vt")
        for j in range(4):
            sl = slice((sb * 4 + j) * P, (sb * 4 + j + 1) * P)
            nc.tensor.transpose(vt_ps[:, j, :], v2f[:, sl], ident_sb[:])
        # vt has a 129th column preset to 1.0 (ones-feature -> denominators)
        vt = vtpool.tile([P, 4, P + 1], BF16, tag="vts", name="vts")
        nc.scalar.copy(out=vt[:, :, 0:P], in_=vt_ps[:])
        for h in range(2):
            o_ps = ps_o.tile([P, 2, P + 1], F32, tag="o", name="o")
            for jj in range(2):
                j = h * 2 + jj
                nc.tensor.matmul(o_ps[:, jj, :], lhsT=e[:, j, :],
                                 rhs=vt[:, j, :], start=True, stop=True)
            rec = recp.tile([P, 2, 1], F32, tag="rec", name="rec")
            nc.vector.reciprocal(out=rec[:], in_=o_ps[:, :, P:P + 1])
            rec_bc = rec[:].broadcast_to([P, 2, P])
            dst = st_ot["t"][:, (sb % 8) * 4 + h * 2:(sb % 8) * 4 + h * 2 + 2, :]
            nc.vector.tensor_tensor(out=dst, in0=o_ps[:, :, 0:P],
                                    in1=rec_bc, op=ALU.mult)
        if sb % 8 == 7:
            t0 = c * NB + (sb - 7) * 4          # first block of this out tile
            nc.gpsimd.dma_start(out=outT[:, t0:t0 + OTB, :], in_=st_ot["t"][:])

    # ---- projections with attention pieces pumped in --------------------
    def load_x(c):
        xt = xpool.tile([P, KT, CH], BF16, tag="xt", name="xt")
        nc.gpsimd.dma_start(out=xt[:], in_=xT[c])
        return xt

    def proj_slot(p, m, xt):
        w = wpool.tile([P, KT, P], BF16, tag="wt", name="wt")
        nc.sync.dma_start(out=w[:], in_=wT[p][m])
        ps = pp_ps.tile([P, CH], F32, tag="pp", name="pp")
        for k in range(KT):
            nc.tensor.matmul(ps[:], lhsT=w[:, k, :], rhs=xt[:, k, :],
                             start=(k == 0), stop=(k == KT - 1))
        dst = asm[p][:, :, m, :]
        src = ps[:].rearrange("p (b s) -> p b s", s=8)
        nc.scalar.activation(out=dst, in_=src, func=AF.Identity,
                             bias=bias_sb[p][:, m:m + 1], scale=1.0)

    xts = {0: load_x(0)}
    for c in range(NCHUNK):
        xt = xts.pop(c)
        pending_b = [lambda sb=sb: piece_b(c - 1, sb) for sb in range(NSB)] \
            if c > 0 else []
        # q,k slots: pump previous chunk's att@v pieces (1 per 2 slots)
        for i, p in enumerate("qk"):
            asm[p] = asmp.tile([P, NB, G, 8], BF16, tag=f"asm{p}",
                               name=f"asm{p}")
            for m in range(MT):
                proj_slot(p, m, xt)
                if (i * MT + m) % 2 == 1 and pending_b:
                    pending_b.pop(0)()
        while pending_b:
            pending_b.pop(0)()
        # v slots: pump this chunk's scores pieces (1 per slot)
        if c + 1 < NCHUNK:
            xts[c + 1] = load_x(c + 1)
        asm["v"] = asmp.tile([P, NB, G, 8], BF16, tag="asmv", name="asmv")
        for m in range(MT):
            proj_slot("v", m, xt)
            if m < NSB:
                piece_a(c, m)
    # drain: att@v of the last chunk
    for sb in range(NSB):
        piece_b(NCHUNK - 1, sb)


_PROGRAM = None


def _build():
    global _PROGRAM
    if _PROGRAM is not None:
        return _PROGRAM
    from contextlib import ExitStack

    nc = bacc.Bacc("TRN2", target_bir_lowering=False, debug=False,
                   num_devices=N_CORES)
    with tile.TileContext(nc) as tc:
        with ExitStack() as ctx:
            _emit(nc, tc, ctx)
    nc.compile()
    _PROGRAM = nc
    return nc


def _host_inputs(x, Wq, bq, Wk, bk, Wv, bv):
    """Build the per-core input maps (host-side shard + transpose + cast)."""
    scale = 1.0 / np.sqrt(DG)
    xf = np.ascontiguousarray(x.reshape(-1, D))           # [16384, D]
    assert xf.shape[0] == N_CORES * TC

    bf = ml_dtypes.bfloat16

    def tile_w(WT):
        # [D_in, D_out] -> [MT, P, KT, P]: contiguous 512KB per m-tile
        a = WT.reshape(KT, P, MT, P).transpose(2, 1, 0, 3)
        return np.ascontiguousarray(a).astype(bf)

    shared = {
        "wqT": tile_w((Wq * scale).T),
        "wkT": tile_w(Wk.T),
        "wvT": tile_w(Wv.T),
        "bq": np.ascontiguousarray((bq * scale).reshape(G, DG).T).astype(np.float32),
        "bk": np.ascontiguousarray(bk.reshape(G, DG).T).astype(np.float32),
        "bv": np.ascontiguousarray(bv.reshape(G, DG).T).astype(np.float32),
        "m01": np.ascontiguousarray(
            np.kron(np.ones((G, G), dtype=np.float32),
                    np.eye(8, dtype=np.float32))).astype(bf),
        "ident": np.eye(P, dtype=np.float32).astype(bf),
    }
    in_maps = []
    for i in range(N_CORES):
        xi = xf[i * TC:(i + 1) * TC]
        m = dict(shared)
        # [TC, D] -> xT tiled [NCHUNK, P, KT, CH]
        xt = xi.T.reshape(KT, P, NCHUNK, CH).transpose(2, 1, 0, 3)
        m["xT"] = np.ascontiguousarray(xt).astype(bf)
        in_maps.append(m)
    return in_maps


last_results = None


def _install_ntff_shim():
    """Provide antenv.axon_hooks if the image lacks it (profiling only)."""
    import sys
    try:
        from antenv.axon_hooks import get_axon_ntff_profile_hook  # noqa: F401
        return
    except ImportError:
        pass
    import contextlib
    import ctypes
    import types

    so_path = "/opt/axon/libaxon_pjrt.so"
    hook = None
    if os.path.exists(so_path):
        lib = ctypes.CDLL(so_path)
        if hasattr(lib, "axon_start_nrt_profile"):
            lib.axon_start_nrt_profile.argtypes = [
                ctypes.POINTER(ctypes.c_int64), ctypes.c_size_t]
            lib.axon_start_nrt_profile.restype = ctypes.c_int64
            lib.axon_stop_nrt_profile.argtypes = [ctypes.c_char_p]
            lib.axon_stop_nrt_profile.restype = ctypes.c_int64

            @contextlib.contextmanager
            def _hook(output_dir, device_ids):
                import jax
                jax.devices()
                if device_ids:
                    ids = (ctypes.c_int64 * len(device_ids))(*device_ids)
                    rc = lib.axon_start_nrt_profile(ids, len(device_ids))
                else:
                    rc = lib.axon_start_nrt_profile(None, 0)
                if rc != 0:
                    raise RuntimeError(f"axon_start_nrt_profile rc={rc}")
                try:
                    yield
                finally:
                    n = lib.axon_stop_nrt_profile(str(output_dir).encode())
                    print(f"profile: {n} file(s) written to {output_dir}")

            hook = _hook

    mod = types.ModuleType("antenv.axon_hooks")
    mod.get_axon_ntff_profile_hook = lambda: hook
    mod.set_axon_ntff_profile_hook = lambda h: None
    import antenv
    antenv.axon_hooks = mod
    sys.modules["antenv.axon_hooks"] = mod


def kernel(**inputs):
    global last_results
    nc = _build()
    in_maps = _host_inputs(**inputs)
    trace = bool(os.environ.get("BASS_TRACE"))
    if trace:
        _install_ntff_shim()
    res = run_bass_kernel_spmd(nc, in_maps, list(range(N_CORES)), trace=trace)
    last_results = res
    x = inputs["x"]
    out = np.empty((N_CORES * TC, D), dtype=np.float32)
    for i in range(N_CORES):
        o = res.results[i]["outT"].astype(np.float32)      # [P, TC/8, P]
        o = o.reshape(G, 8, TC // 8, DG).transpose(2, 1, 0, 3)
        out[i * TC:(i + 1) * TC] = o.reshape(TC, D)
    return out.reshape(x.shape)


# revision 10
# speedup vs baseline: 1.0769x; 1.0066x over previous
"""Trainium2 Bass kernel for per-token grouped attention (GQA-style).

Computation (per token t):
    q = x @ Wq.T + bq ; k = x @ Wk.T + bk ; v = x @ Wv.T + bv     (D=2048)
    reshape to (G=16 groups, d=128); scores = q_g . k_h / sqrt(d) (16x16)
    att = softmax(scores, axis=h); out = att @ v  -> (G*d,)

Sharding: data-parallel over the B*T = 16384 tokens across 8 cores
(2048 tokens/core).  Device works feature-major for the projections; the
attention emits the output token-major ([(g,s), block, dd]) and the host
unscrambles.

Device program (per core, SPMD), 4 chunks of 512 tokens:
  Projections: qT/kT/vT = W.T-tiles @ xT, bf16 matmuls with fp32 PSUM
    accumulation, bias added during the PSUM->SBUF copy (ACT), scattered
    into block-interleaved SBUF tiles [dd, block, g, s] (single-buffered).
  Attention per 8-token block b (3.01 matmul-equivalents instead of 4):
    sT = k_blk^T q_blk  (one 128x128 MM: all 64 pairwise 16x16 tiles,
         only the 8 diagonal ones survive the mask)
    e  = exp(sT) * blockdiag-mask          (ACT + DVE)
    vT = PE-transpose(v_blk)               (1 MM)
    o  = e^T @ vT   -> out^T[(g,s), dd]    (1 MM, unnormalized)
    dn = e^T @ ones -> softmax denominators (1-column MM, ~free)
    out = o * (1/dn) broadcast             (DVE, batched per super-block)
  Pump schedule (keeps PE fed, allows single-buffered q/k/v tiles):
    chunk c's q,k slots <- att@v pieces of chunk c-1
    chunk c's v slots   <- scores pieces of chunk c
    after last chunk    <- drain att@v of last chunk
"""

import os
import numpy as np
import ml_dtypes

import concourse.bass as bass
import concourse.tile as tile
from concourse import bacc, mybir
from concourse.bass_utils import run_bass_kernel_spmd

F32 = mybir.dt.float32
BF16 = mybir.dt.bfloat16
AF = mybir.ActivationFunctionType
ALU = mybir.AluOpType

P = 128          # SBUF partitions
D = 2048         # model dim
G = 16           # groups
DG = 128         # per-group dim
N_CORES = 8
TC = 2048        # tokens per core
NCHUNK = 4
CH = TC // NCHUNK          # 512 tokens per chunk
NB = CH // 8               # 64 blocks of 8 tokens per chunk
NSB = NB // 4              # 16 super-blocks (32 tokens) per chunk
KT = D // P      # 16 contraction tiles
MT = D // P      # 16 output-feature tiles
OTB = 32         # blocks per output tile (256 tokens)


def _emit(nc, tc, ctx):
    # ---- DRAM I/O -------------------------------------------------------
    xT = nc.dram_tensor("xT", [NCHUNK, P, KT, CH], BF16,
                        kind="ExternalInput").ap()
    wT = {
        p: nc.dram_tensor(f"w{p}T", [MT, P, KT, P], BF16,
                          kind="ExternalInput").ap()
        for p in "qkv"
    }
    b_dram = nc.dram_tensor("bqkv", [P, 3, G], F32, kind="ExternalInput").ap()
    mi_dram = nc.dram_tensor("m01ident", [P, 2, P], BF16,
                             kind="ExternalInput").ap()
    outT = nc.dram_tensor("outT", [P, TC // 8, P], BF16,
                          kind="ExternalOutput").ap()

    # ---- pools ----------------------------------------------------------
    singles = ctx.enter_context(tc.tile_pool(name="singles", bufs=1))
    xpool = ctx.enter_context(tc.tile_pool(name="xpool", bufs=2))
    wpool = ctx.enter_context(tc.tile_pool(name="wpool", bufs=6))
    asmp = ctx.enter_context(tc.tile_pool(name="asmp", bufs=1))
    epool = ctx.enter_context(tc.tile_pool(name="epool", bufs=NSB + 2))
    vtpool = ctx.enter_context(tc.tile_pool(name="vtpool", bufs=3))
    recp = ctx.enter_context(tc.tile_pool(name="recp", bufs=3))
    otp = ctx.enter_context(tc.tile_pool(name="otp", bufs=2))

    pp_ps = ctx.enter_context(tc.tile_pool(name="pp_ps", bufs=2, space="PSUM"))
    ps_s = ctx.enter_context(tc.tile_pool(name="ps_s", bufs=1, space="PSUM"))
    ps_vt = ctx.enter_context(tc.tile_pool(name="ps_vt", bufs=2, space="PSUM"))
    ps_o = ctx.enter_context(tc.tile_pool(name="ps_o", bufs=3, space="PSUM"))

    # ---- constants (keep the SP/sync queue free for weight tiles) -------
    ball = singles.tile([P, 3, G], F32, tag="bias", name="bias")
    nc.scalar.dma_start(out=ball[:], in_=b_dram[:])
    bias_sb = {p: ball[:, i, :] for i, p in enumerate("qkv")}
    mi_sb = singles.tile([P, 2, P], BF16, tag="mi", name="mi")
    m01_sb = mi_sb[:, 0, :]
    ident_sb = mi_sb[:, 1, :]

    # pre-warm the vt ring: the 129th column stays 1.0 forever (the
    # ones-feature that makes att@v emit softmax denominators in column P)
    for _ in range(3):
        vtw = vtpool.tile([P, 4, P + 1], BF16, tag="vts", name="vts")
        nc.vector.memset(vtw[:, :, P:P + 1], 1.0)

    # per-chunk assembled q/k/v (block-interleaved [dd, block, g, s]),
    # single-buffered: the pump schedule guarantees producer/consumer order.
    asm = {}

    # ---- attention pieces ----------------------------------------------
    st_e = {}       # (sb) -> masked exp tile for current chunk's scores
    st_ot = {}      # out tile in progress

    def piece_a(c, sb):
        """Scores + exp + mask for super-block sb of chunk c."""
        q2f = asm["q"].rearrange("p b g s -> p (b g s)")
        k2f = asm["k"].rearrange("p b g s -> p (b g s)")
        sT = ps_s.tile([P, 4, P], F32, tag="s", name="s")
        for j in range(4):
            sl = slice((sb * 4 + j) * P, (sb * 4 + j + 1) * P)
            nc.tensor.matmul(sT[:, j, :], lhsT=k2f[:, sl], rhs=q2f[:, sl],
                             start=True, stop=True)
        e = epool.tile([P, 4, P], BF16, tag="e", name="e")
        nc.scalar.activation(out=e[:], in_=sT[:], func=AF.Exp)
        m01_bc = m01_sb.unsqueeze(1).broadcast_to([P, 4, P])
        nc.vector.tensor_tensor(out=e[:], in0=e[:], in1=m01_bc, op=ALU.mult)
        st_e[sb] = e

    def piece_b(c, sb):
        """v-transpose + att@v (with fused denominators) + normalize."""
        if sb % 8 == 0:
            st_ot["t"] = otp.tile([P, OTB, P], BF16, tag="ot", name="ot")
        e = st_e.pop(sb)
        v2f = asm["v"].rearrange("p b g s -> p (b g s)")
        vt_ps = ps_vt.tile([P, 4, P], BF16, tag="vt", name="vt")
        for j in range(4):
            sl = slice((sb * 4 + j) * P, (sb * 4 + j + 1) * P)
            nc.tensor.transpose(vt_ps[:, j, :], v2f[:, sl], ident_sb)
        # vt has a 129th column preset to 1.0 (ones-feature -> denominators)
        vt = vtpool.tile([P, 4, P + 1], BF16, tag="vts", name="vts")
        nc.scalar.copy(out=vt[:, :, 0:P], in_=vt_ps[:])
        for h in range(2):
            o_ps = ps_o.tile([P, 2, P + 1], F32, tag="o", name="o")
            for jj in range(2):
                j = h * 2 + jj
                nc.tensor.matmul(o_ps[:, jj, :], lhsT=e[:, j, :],
                                 rhs=vt[:, j, :], start=True, stop=True)
            rec = recp.tile([P, 2, 1], F32, tag="rec", name="rec")
            nc.vector.reciprocal(out=rec[:], in_=o_ps[:, :, P:P + 1])
            rec_bc = rec[:].broadcast_to([P, 2, P])
            dst = st_ot["t"][:, (sb % 8) * 4 + h * 2:(sb % 8) * 4 + h * 2 + 2, :]
            nc.vector.tensor_tensor(out=dst, in0=o_ps[:, :, 0:P],
                                    in1=rec_bc, op=ALU.mult)
        if sb % 8 == 7:
            t0 = c * NB + (sb - 7) * 4          # first block of this out tile
            nc.gpsimd.dma_start(out=outT[:, t0:t0 + OTB, :], in_=st_ot["t"][:])

    # ---- projections with attention pieces pumped in --------------------
    def load_x(c):
        xt = xpool.tile([P, KT, CH], BF16, tag="xt", name="xt")
        nc.gpsimd.dma_start(out=xt[:], in_=xT[c])
        return xt

    def proj_slot(p, m, xt):
        w = wpool.tile([P, KT, P], BF16, tag="wt", name="wt")
        nc.sync.dma_start(out=w[:], in_=wT[p][m])
        ps = pp_ps.tile([P, CH], F32, tag="pp", name="pp")
        for k in range(KT):
            nc.tensor.matmul(ps[:], lhsT=w[:, k, :], rhs=xt[:, k, :],
                             start=(k == 0), stop=(k == KT - 1))
        dst = asm[p][:, :, m, :]
        src = ps[:].rearrange("p (b s) -> p b s", s=8)
        nc.scalar.activation(out=dst, in_=src, func=AF.Identity,
                             bias=bias_sb[p][:, m:m + 1], scale=1.0)

    xts = {0: load_x(0)}
    for c in range(NCHUNK):
        xt = xts.pop(c)
        pending_b = [lambda sb=sb: piece_b(c - 1, sb) for sb in range(NSB)] \
            if c > 0 else []
        # q,k slots: pump previous chunk's att@v pieces (1 per 2 slots)
        for i, p in enumerate("qk"):
            asm[p] = asmp.tile([P, NB, G, 8], BF16, tag=f"asm{p}",
                               name=f"asm{p}")
            for m in range(MT):
                proj_slot(p, m, xt)
                if (i * MT + m) % 2 == 1 and pending_b:
                    pending_b.pop(0)()
        while pending_b:
            pending_b.pop(0)()
        if c == 0:
            nc.scalar.dma_start(out=mi_sb[:], in_=mi_dram[:])
        # v slots: pump this chunk's scores pieces (1 per slot)
        if c + 1 < NCHUNK:
            xts[c + 1] = load_x(c + 1)
        asm["v"] = asmp.tile([P, NB, G, 8], BF16, tag="asmv", name="asmv")
        for m in range(MT):
            proj_slot("v", m, xt)
            if m < NSB:
                piece_a(c, m)
    # drain: att@v of the last chunk
    for sb in range(NSB):
        piece_b(NCHUNK - 1, sb)


_PROGRAM = None


def _build():
    global _PROGRAM
    if _PROGRAM is not None:
        return _PROGRAM
    from contextlib import ExitStack

    nc = bacc.Bacc("TRN2", target_bir_lowering=False, debug=False,
                   num_devices=N_CORES)
    with tile.TileContext(nc) as tc:
        with ExitStack() as ctx:
            _emit(nc, tc, ctx)
    nc.compile()
    _PROGRAM = nc
    return nc


def _host_inputs(x, Wq, bq, Wk, bk, Wv, bv):
    """Build the per-core input maps (host-side shard + transpose + cast)."""
    scale = 1.0 / np.sqrt(DG)
    xf = np.ascontiguousarray(x.reshape(-1, D))           # [16384, D]
    assert xf.shape[0] == N_CORES * TC

    bf = ml_dtypes.bfloat16

    def tile_w(WT):
        # [D_in, D_out] -> [MT, P, KT, P]: contiguous 512KB per m-tile
        a = WT.reshape(KT, P, MT, P).transpose(2, 1, 0, 3)
        return np.ascontiguousarray(a).astype(bf)

    shared = {
        "wqT": tile_w((Wq * scale).T),
        "wkT": tile_w(Wk.T),
        "wvT": tile_w(Wv.T),
        "bqkv": np.ascontiguousarray(np.stack([
            (bq * scale).reshape(G, DG).T,
            bk.reshape(G, DG).T,
            bv.reshape(G, DG).T], axis=1)).astype(np.float32),
        "m01ident": np.ascontiguousarray(np.stack([
            np.kron(np.ones((G, G), dtype=np.float32),
                    np.eye(8, dtype=np.float32)),
            np.eye(P, dtype=np.float32)], axis=1)).astype(bf),
    }
    in_maps = []
    for i in range(N_CORES):
        xi = xf[i * TC:(i + 1) * TC]
        m = dict(shared)
        # [TC, D] -> xT tiled [NCHUNK, P, KT, CH]
        xt = xi.T.reshape(KT, P, NCHUNK, CH).transpose(2, 1, 0, 3)
        m["xT"] = np.ascontiguousarray(xt).astype(bf)
        in_maps.append(m)
    return in_maps


last_results = None


def _install_ntff_shim():
    """Provide antenv.axon_hooks if the image lacks it (profiling only)."""
    import sys
    try:
        from antenv.axon_hooks import get_axon_ntff_profile_hook  # noqa: F401
        return
    except ImportError:
        pass
    import contextlib
    import ctypes
    import types

    so_path = "/opt/axon/libaxon_pjrt.so"
    hook = None
    if os.path.exists(so_path):
        lib = ctypes.CDLL(so_path)
        if hasattr(lib, "axon_start_nrt_profile"):
            lib.axon_start_nrt_profile.argtypes = [
                ctypes.POINTER(ctypes.c_int64), ctypes.c_size_t]
            lib.axon_start_nrt_profile.restype = ctypes.c_int64
            lib.axon_stop_nrt_profile.argtypes = [ctypes.c_char_p]
            lib.axon_stop_nrt_profile.restype = ctypes.c_int64

            @contextlib.contextmanager
            def _hook(output_dir, device_ids):
                import jax
                jax.devices()
                if device_ids:
                    ids = (ctypes.c_int64 * len(device_ids))(*device_ids)
                    rc = lib.axon_start_nrt_profile(ids, len(device_ids))
                else:
                    rc = lib.axon_start_nrt_profile(None, 0)
                if rc != 0:
                    raise RuntimeError(f"axon_start_nrt_profile rc={rc}")
                try:
                    yield
                finally:
                    n = lib.axon_stop_nrt_profile(str(output_dir).encode())
                    print(f"profile: {n} file(s) written to {output_dir}")

            hook = _hook

    mod = types.ModuleType("antenv.axon_hooks")
    mod.get_axon_ntff_profile_hook = lambda: hook
    mod.set_axon_ntff_profile_hook = lambda h: None
    import antenv
    antenv.axon_hooks = mod
    sys.modules["antenv.axon_hooks"] = mod


def kernel(**inputs):
    global last_results
    nc = _build()
    in_maps = _host_inputs(**inputs)
    trace = bool(os.environ.get("BASS_TRACE"))
    if trace:
        _install_ntff_shim()
    res = run_bass_kernel_spmd(nc, in_maps, list(range(N_CORES)), trace=trace)
    last_results = res
    x = inputs["x"]
    out = np.empty((N_CORES * TC, D), dtype=np.float32)
    for i in range(N_CORES):
        o = res.results[i]["outT"].astype(np.float32)      # [P, TC/8, P]
        o = o.reshape(G, 8, TC // 8, DG).transpose(2, 1, 0, 3)
        out[i * TC:(i + 1) * TC] = o.reshape(TC, D)
    return out.reshape(x.shape)


# revision 11
# speedup vs baseline: 1.0827x; 1.0054x over previous
"""Trainium2 Bass kernel for per-token grouped attention (GQA-style).

Computation (per token t):
    q = x @ Wq.T + bq ; k = x @ Wk.T + bk ; v = x @ Wv.T + bv     (D=2048)
    reshape to (G=16 groups, d=128); scores = q_g . k_h / sqrt(d) (16x16)
    att = softmax(scores, axis=h); out = att @ v  -> (G*d,)

Sharding: data-parallel over the B*T = 16384 tokens across 8 cores
(2048 tokens/core).  Device works feature-major for the projections; the
attention emits the output token-major ([(g,s), block, dd]) and the host
unscrambles.

Device program (per core, SPMD), 4 chunks of 512 tokens:
  Projections: qT/kT/vT = W.T-tiles @ xT, bf16 matmuls with fp32 PSUM
    accumulation, bias added during the PSUM->SBUF copy (ACT), scattered
    into block-interleaved SBUF tiles [dd, block, g, s] (single-buffered).
  Attention per 8-token block b (3.01 matmul-equivalents instead of 4):
    sT = k_blk^T q_blk  (one 128x128 MM: all 64 pairwise 16x16 tiles,
         only the 8 diagonal ones survive the mask)
    e  = exp(sT) * blockdiag-mask          (ACT + DVE)
    vT = PE-transpose(v_blk)               (1 MM)
    o  = e^T @ vT   -> out^T[(g,s), dd]    (1 MM, unnormalized)
    dn = e^T @ ones -> softmax denominators (1-column MM, ~free)
    out = o * (1/dn) broadcast             (DVE, batched per super-block)
  Pump schedule (keeps PE fed, allows single-buffered q/k/v tiles):
    chunk c's q,k slots <- att@v pieces of chunk c-1
    chunk c's v slots   <- scores pieces of chunk c
    after last chunk    <- drain att@v of last chunk
"""

import os
import numpy as np
import ml_dtypes

import concourse.bass as bass
import concourse.tile as tile
from concourse import bacc, mybir
from concourse.bass_utils import run_bass_kernel_spmd

F32 = mybir.dt.float32
BF16 = mybir.dt.bfloat16
AF = mybir.ActivationFunctionType
ALU = mybir.AluOpType

P = 128          # SBUF partitions
D = 2048         # model dim
G = 16           # groups
DG = 128         # per-group dim
N_CORES = 8
TC = 2048        # tokens per core
NCHUNK = 4
CH = TC // NCHUNK          # 512 tokens per chunk
NB = CH // 8               # 64 blocks of 8 tokens per chunk
NSB = NB // 4              # 16 super-blocks (32 tokens) per chunk
KT = D // P      # 16 contraction tiles
MT = D // P      # 16 output-feature tiles
OTB = 16         # blocks per output tile (128 tokens)


def _emit(nc, tc, ctx):
    # ---- DRAM I/O -------------------------------------------------------
    xT = nc.dram_tensor("xT", [NCHUNK, P, KT, CH], BF16,
                        kind="ExternalInput").ap()
    wT = {
        p: nc.dram_tensor(f"w{p}T", [MT, P, KT, P], BF16,
                          kind="ExternalInput").ap()
        for p in "qkv"
    }
    b_dram = nc.dram_tensor("bqkv", [P, 3, G], F32, kind="ExternalInput").ap()
    mi_dram = nc.dram_tensor("m01ident", [P, 2, P], BF16,
                             kind="ExternalInput").ap()
    outT = nc.dram_tensor("outT", [P, TC // 8, P], BF16,
                          kind="ExternalOutput").ap()

    # ---- pools ----------------------------------------------------------
    singles = ctx.enter_context(tc.tile_pool(name="singles", bufs=1))
    xpool = ctx.enter_context(tc.tile_pool(name="xpool", bufs=2))
    wpool = ctx.enter_context(tc.tile_pool(name="wpool", bufs=6))
    asmp = ctx.enter_context(tc.tile_pool(name="asmp", bufs=1))
    epool = ctx.enter_context(tc.tile_pool(name="epool", bufs=NSB + 2))
    vtpool = ctx.enter_context(tc.tile_pool(name="vtpool", bufs=3))
    recp = ctx.enter_context(tc.tile_pool(name="recp", bufs=3))
    otp = ctx.enter_context(tc.tile_pool(name="otp", bufs=2))

    pp_ps = ctx.enter_context(tc.tile_pool(name="pp_ps", bufs=2, space="PSUM"))
    ps_s = ctx.enter_context(tc.tile_pool(name="ps_s", bufs=1, space="PSUM"))
    ps_vt = ctx.enter_context(tc.tile_pool(name="ps_vt", bufs=2, space="PSUM"))
    ps_o = ctx.enter_context(tc.tile_pool(name="ps_o", bufs=3, space="PSUM"))

    # ---- constants (keep the SP/sync queue free for weight tiles) -------
    ball = singles.tile([P, 3, G], F32, tag="bias", name="bias")
    nc.scalar.dma_start(out=ball[:], in_=b_dram[:])
    bias_sb = {p: ball[:, i, :] for i, p in enumerate("qkv")}
    mi_sb = singles.tile([P, 2, P], BF16, tag="mi", name="mi")
    m01_sb = mi_sb[:, 0, :]
    ident_sb = mi_sb[:, 1, :]

    # pre-warm the vt ring: the 129th column stays 1.0 forever (the
    # ones-feature that makes att@v emit softmax denominators in column P)
    for _ in range(3):
        vtw = vtpool.tile([P, 4, P + 1], BF16, tag="vts", name="vts")
        nc.vector.memset(vtw[:, :, P:P + 1], 1.0)

    # per-chunk assembled q/k/v (block-interleaved [dd, block, g, s]),
    # single-buffered: the pump schedule guarantees producer/consumer order.
    asm = {}

    # ---- attention pieces ----------------------------------------------
    st_e = {}       # (sb) -> masked exp tile for current chunk's scores
    st_vt = {}      # (sb) -> transposed-v SBUF tile (stage b_t -> b_m)
    st_ot = {}      # out tile in progress

    def piece_a(c, sb):
        """Scores + exp + mask for super-block sb of chunk c."""
        q2f = asm["q"].rearrange("p b g s -> p (b g s)")
        k2f = asm["k"].rearrange("p b g s -> p (b g s)")
        sT = ps_s.tile([P, 4, P], F32, tag="s", name="s")
        for j in range(4):
            sl = slice((sb * 4 + j) * P, (sb * 4 + j + 1) * P)
            nc.tensor.matmul(sT[:, j, :], lhsT=k2f[:, sl], rhs=q2f[:, sl],
                             start=True, stop=True)
        e = epool.tile([P, 4, P], BF16, tag="e", name="e")
        nc.scalar.activation(out=e[:], in_=sT[:], func=AF.Exp)
        m01_bc = m01_sb.unsqueeze(1).broadcast_to([P, 4, P])
        nc.vector.tensor_tensor(out=e[:], in0=e[:], in1=m01_bc, op=ALU.mult)
        st_e[sb] = e

    def piece_bt(c, sb):
        """Stage 1: v-transpose + PSUM->SBUF copy (with ones column)."""
        v2f = asm["v"].rearrange("p b g s -> p (b g s)")
        vt_ps = ps_vt.tile([P, 4, P], BF16, tag="vt", name="vt")
        for j in range(4):
            sl = slice((sb * 4 + j) * P, (sb * 4 + j + 1) * P)
            nc.tensor.transpose(vt_ps[:, j, :], v2f[:, sl], ident_sb)
        # vt has a 129th column preset to 1.0 (ones-feature -> denominators)
        vt = vtpool.tile([P, 4, P + 1], BF16, tag="vts", name="vts")
        nc.scalar.copy(out=vt[:, :, 0:P], in_=vt_ps[:])
        st_vt[sb] = vt

    def piece_bm(c, sb):
        """Stage 2: att@v (with fused denominators) + normalize + store."""
        if sb % 4 == 0:
            st_ot["t"] = otp.tile([P, OTB, P], BF16, tag="ot", name="ot")
        e = st_e.pop(sb)
        vt = st_vt.pop(sb)
        for h in range(2):
            o_ps = ps_o.tile([P, 2, P + 1], F32, tag="o", name="o")
            for jj in range(2):
                j = h * 2 + jj
                nc.tensor.matmul(o_ps[:, jj, :], lhsT=e[:, j, :],
                                 rhs=vt[:, j, :], start=True, stop=True)
            rec = recp.tile([P, 2, 1], F32, tag="rec", name="rec")
            nc.vector.reciprocal(out=rec[:], in_=o_ps[:, :, P:P + 1])
            rec_bc = rec[:].broadcast_to([P, 2, P])
            dst = st_ot["t"][:, (sb % 4) * 4 + h * 2:(sb % 4) * 4 + h * 2 + 2, :]
            nc.vector.tensor_tensor(out=dst, in0=o_ps[:, :, 0:P],
                                    in1=rec_bc, op=ALU.mult)
        if sb % 4 == 3:
            t0 = c * NB + (sb - 3) * 4          # first block of this out tile
            nc.gpsimd.dma_start(out=outT[:, t0:t0 + OTB, :], in_=st_ot["t"][:])

    # ---- projections with attention pieces pumped in --------------------
    def load_x(c):
        xt = xpool.tile([P, KT, CH], BF16, tag="xt", name="xt")
        nc.gpsimd.dma_start(out=xt[:], in_=xT[c])
        return xt

    def proj_slot(p, m, xt):
        w = wpool.tile([P, KT, P], BF16, tag="wt", name="wt")
        nc.sync.dma_start(out=w[:], in_=wT[p][m])
        ps = pp_ps.tile([P, CH], F32, tag="pp", name="pp")
        for k in range(KT):
            nc.tensor.matmul(ps[:], lhsT=w[:, k, :], rhs=xt[:, k, :],
                             start=(k == 0), stop=(k == KT - 1))
        dst = asm[p][:, :, m, :]
        src = ps[:].rearrange("p (b s) -> p b s", s=8)
        nc.scalar.activation(out=dst, in_=src, func=AF.Identity,
                             bias=bias_sb[p][:, m:m + 1], scale=1.0)

    xts = {0: load_x(0)}
    for c in range(NCHUNK):
        xt = xts.pop(c)
        pending_b = []
        if c > 0:
            bt = [lambda sb=sb: piece_bt(c - 1, sb) for sb in range(NSB)]
            bm = [lambda sb=sb: piece_bm(c - 1, sb) for sb in range(NSB)]
            pending_b = [bt[0]]
            for sb in range(NSB):
                if sb + 1 < NSB:
                    pending_b.append(bt[sb + 1])
                pending_b.append(bm[sb])
        # q,k slots: pump previous chunk's att@v pieces (1 per slot)
        for i, p in enumerate("qk"):
            asm[p] = asmp.tile([P, NB, G, 8], BF16, tag=f"asm{p}",
                               name=f"asm{p}")
            for m in range(MT):
                proj_slot(p, m, xt)
                if pending_b:
                    pending_b.pop(0)()
        while pending_b:
            pending_b.pop(0)()
        if c == 0:
            nc.scalar.dma_start(out=mi_sb[:], in_=mi_dram[:])
        # v slots: pump this chunk's scores pieces (1 per slot)
        if c + 1 < NCHUNK:
            xts[c + 1] = load_x(c + 1)
        asm["v"] = asmp.tile([P, NB, G, 8], BF16, tag="asmv", name="asmv")
        for m in range(MT):
            proj_slot("v", m, xt)
            if m < NSB:
                piece_a(c, m)
    # drain: att@v of the last chunk, transposes one step ahead
    piece_bt(NCHUNK - 1, 0)
    for sb in range(NSB):
        if sb + 1 < NSB:
            piece_bt(NCHUNK - 1, sb + 1)
        piece_bm(NCHUNK - 1, sb)


_PROGRAM = None


def _build():
    global _PROGRAM
    if _PROGRAM is not None:
        return _PROGRAM
    from contextlib import ExitStack

    nc = bacc.Bacc("TRN2", target_bir_lowering=False, debug=False,
                   num_devices=N_CORES)
    with tile.TileContext(nc) as tc:
        with ExitStack() as ctx:
            _emit(nc, tc, ctx)
    nc.compile()
    _PROGRAM = nc
    return nc


def _host_inputs(x, Wq, bq, Wk, bk, Wv, bv):
    """Build the per-core input maps (host-side shard + transpose + cast)."""
    scale = 1.0 / np.sqrt(DG)
    xf = np.ascontiguousarray(x.reshape(-1, D))           # [16384, D]
    assert xf.shape[0] == N_CORES * TC

    bf = ml_dtypes.bfloat16

    def tile_w(WT):
        # [D_in, D_out] -> [MT, P, KT, P]: contiguous 512KB per m-tile
        a = WT.reshape(KT, P, MT, P).transpose(2, 1, 0, 3)
        return np.ascontiguousarray(a).astype(bf)

    shared = {
        "wqT": tile_w((Wq * scale).T),
        "wkT": tile_w(Wk.T),
        "wvT": tile_w(Wv.T),
        "bqkv": np.ascontiguousarray(np.stack([
            (bq * scale).reshape(G, DG).T,
            bk.reshape(G, DG).T,
            bv.reshape(G, DG).T], axis=1)).astype(np.float32),
        "m01ident": np.ascontiguousarray(np.stack([
            np.kron(np.ones((G, G), dtype=np.float32),
                    np.eye(8, dtype=np.float32)),
            np.eye(P, dtype=np.float32)], axis=1)).astype(bf),
    }
    in_maps = []
    for i in range(N_CORES):
        xi = xf[i * TC:(i + 1) * TC]
        m = dict(shared)
        # [TC, D] -> xT tiled [NCHUNK, P, KT, CH]
        xt = xi.T.reshape(KT, P, NCHUNK, CH).transpose(2, 1, 0, 3)
        m["xT"] = np.ascontiguousarray(xt).astype(bf)
        in_maps.append(m)
    return in_maps


last_results = None


def _install_ntff_shim():
    """Provide antenv.axon_hooks if the image lacks it (profiling only)."""
    import sys
    try:
        from antenv.axon_hooks import get_axon_ntff_profile_hook  # noqa: F401
        return
    except ImportError:
        pass
    import contextlib
    import ctypes
    import types

    so_path = "/opt/axon/libaxon_pjrt.so"
    hook = None
    if os.path.exists(so_path):
        lib = ctypes.CDLL(so_path)
        if hasattr(lib, "axon_start_nrt_profile"):
            lib.axon_start_nrt_profile.argtypes = [
                ctypes.POINTER(ctypes.c_int64), ctypes.c_size_t]
            lib.axon_start_nrt_profile.restype = ctypes.c_int64
            lib.axon_stop_nrt_profile.argtypes = [ctypes.c_char_p]
            lib.axon_stop_nrt_profile.restype = ctypes.c_int64

            @contextlib.contextmanager
            def _hook(output_dir, device_ids):
                import jax
                jax.devices()
                if device_ids:
                    ids = (ctypes.c_int64 * len(device_ids))(*device_ids)
                    rc = lib.axon_start_nrt_profile(ids, len(device_ids))
                else:
                    rc = lib.axon_start_nrt_profile(None, 0)
                if rc != 0:
                    raise RuntimeError(f"axon_start_nrt_profile rc={rc}")
                try:
                    yield
                finally:
                    n = lib.axon_stop_nrt_profile(str(output_dir).encode())
                    print(f"profile: {n} file(s) written to {output_dir}")

            hook = _hook

    mod = types.ModuleType("antenv.axon_hooks")
    mod.get_axon_ntff_profile_hook = lambda: hook
    mod.set_axon_ntff_profile_hook = lambda h: None
    import antenv
    antenv.axon_hooks = mod
    sys.modules["antenv.axon_hooks"] = mod


def kernel(**inputs):
    global last_results
    nc = _build()
    in_maps = _host_inputs(**inputs)
    trace = bool(os.environ.get("BASS_TRACE"))
    if trace:
        _install_ntff_shim()
    res = run_bass_kernel_spmd(nc, in_maps, list(range(N_CORES)), trace=trace)
    last_results = res
    x = inputs["x"]
    out = np.empty((N_CORES * TC, D), dtype=np.float32)
    for i in range(N_CORES):
        o = res.results[i]["outT"].astype(np.float32)      # [P, TC/8, P]
        o = o.reshape(G, 8, TC // 8, DG).transpose(2, 1, 0, 3)
        out[i * TC:(i + 1) * TC] = o.reshape(TC, D)
    return out.reshape(x.shape)


# revision 15
# speedup vs baseline: 1.2644x; 1.1678x over previous
"""Trainium2 Bass kernel for per-token grouped attention (GQA-style).

Computation (per token t):
    q = x @ Wq.T + bq ; k = x @ Wk.T + bk ; v = x @ Wv.T + bv     (D=2048)
    reshape to (G=16 groups, d=128); scores = q_g . k_h / sqrt(d) (16x16)
    att = softmax(scores, axis=h); out = att @ v  -> (G*d,)

Sharding: data-parallel over the B*T = 16384 tokens across 8 cores
(2048 tokens/core).  Device works feature-major for the projections; the
attention emits the output token-major ([(g,s), block, dd]) and the host
unscrambles.

Device program (per core, SPMD), 4 chunks of 512 tokens:
  Projections: qT/kT/vT = W.T-tiles @ xT, bf16 matmuls with fp32 PSUM
    accumulation, bias added during the PSUM->SBUF copy (ACT), scattered
    into block-interleaved SBUF tiles [dd, block, g, s] (single-buffered).
  Attention per 8-token block b (3.01 matmul-equivalents instead of 4):
    sT = k_blk^T q_blk  (one 128x128 MM: all 64 pairwise 16x16 tiles,
         only the 8 diagonal ones survive the mask)
    e  = exp(sT) * blockdiag-mask          (ACT + DVE)
    vT = PE-transpose(v_blk)               (1 MM)
    o  = e^T @ vT   -> out^T[(g,s), dd]    (1 MM, unnormalized)
    dn = e^T @ ones -> softmax denominators (1-column MM, ~free)
    out = o * (1/dn) broadcast             (DVE, batched per super-block)
  Pump schedule (keeps PE fed, allows single-buffered q/k/v tiles):
    chunk c's q,k slots <- att@v pieces of chunk c-1
    chunk c's v slots   <- scores pieces of chunk c
    after last chunk    <- drain att@v of last chunk
"""

import os
import numpy as np
import ml_dtypes

import concourse.bass as bass
import concourse.tile as tile
from concourse import bacc, mybir
from concourse.bass_utils import run_bass_kernel_spmd

F32 = mybir.dt.float32
BF16 = mybir.dt.bfloat16
FP8 = mybir.dt.float8e4
AF = mybir.ActivationFunctionType
ALU = mybir.AluOpType

P = 128          # SBUF partitions
D = 2048         # model dim
G = 16           # groups
DG = 128         # per-group dim
N_CORES = 8
TC = 2048        # tokens per core
NCHUNK = 4
CH = TC // NCHUNK          # 512 tokens per chunk
NB = CH // 8               # 64 blocks of 8 tokens per chunk
NSB = NB // 4              # 16 super-blocks (32 tokens) per chunk
KT = D // P      # 16 contraction tiles
KA = KT // 2     # bf16 k-tiles (features 0..1023)
KB = KT // 2     # fp8 k-tiles (features 1024..2047), as KB//2 DoubleRow pairs
MT = D // P      # 16 output-feature tiles
SCL = 2.0 ** -17  # undo the x*16 / W*8192 scaling in the PSUM->SBUF copy
OTB = 16         # blocks per output tile (128 tokens)


def _emit(nc, tc, ctx):
    # ---- DRAM I/O -------------------------------------------------------
    xTa = nc.dram_tensor("xTa", [NCHUNK, P, KA, CH], BF16,
                         kind="ExternalInput").ap()
    xTb8 = nc.dram_tensor("xTb8", [NCHUNK, P, KB, CH], FP8,
                          kind="ExternalInput").ap()
    xTbb = nc.dram_tensor("xTbb", [NCHUNK, P, KB, CH], BF16,
                          kind="ExternalInput").ap()
    wA = {
        p: nc.dram_tensor(f"w{p}A", [MT, P, KA, P], BF16,
                          kind="ExternalInput").ap()
        for p in "qk"
    }
    wB = {
        p: nc.dram_tensor(f"w{p}B", [MT, P, KB // 2, 2, P], FP8,
                          kind="ExternalInput").ap()
        for p in "qk"
    }
    wV = nc.dram_tensor("wvT", [MT, P, KT, P], BF16,
                        kind="ExternalInput").ap()
    b_dram = nc.dram_tensor("bqkv", [P, 3, G], F32, kind="ExternalInput").ap()
    mi_dram = nc.dram_tensor("m01ident", [P, 2, P], BF16,
                             kind="ExternalInput").ap()
    outT = nc.dram_tensor("outT", [P, TC // 8, P], BF16,
                          kind="ExternalOutput").ap()

    # ---- pools ----------------------------------------------------------
    singles = ctx.enter_context(tc.tile_pool(name="singles", bufs=1))
    xpool = ctx.enter_context(tc.tile_pool(name="xpool", bufs=2))
    wpool = ctx.enter_context(tc.tile_pool(name="wpool", bufs=6))
    wpa = ctx.enter_context(tc.tile_pool(name="wpa", bufs=6))
    wpb = ctx.enter_context(tc.tile_pool(name="wpb", bufs=6))
    asmp = ctx.enter_context(tc.tile_pool(name="asmp", bufs=1))
    epool = ctx.enter_context(tc.tile_pool(name="epool", bufs=NSB + 2))
    vtpool = ctx.enter_context(tc.tile_pool(name="vtpool", bufs=3))
    recp = ctx.enter_context(tc.tile_pool(name="recp", bufs=3))
    otp = ctx.enter_context(tc.tile_pool(name="otp", bufs=2))

    pp_ps = ctx.enter_context(tc.tile_pool(name="pp_ps", bufs=2, space="PSUM"))
    ps_s = ctx.enter_context(tc.tile_pool(name="ps_s", bufs=1, space="PSUM"))
    ps_vt = ctx.enter_context(tc.tile_pool(name="ps_vt", bufs=2, space="PSUM"))
    ps_o = ctx.enter_context(tc.tile_pool(name="ps_o", bufs=3, space="PSUM"))

    # ---- constants (keep the SP/sync queue free for weight tiles) -------
    ball = singles.tile([P, 3, G], F32, tag="bias", name="bias")
    nc.scalar.dma_start(out=ball[:], in_=b_dram[:])
    bias_sb = {p: ball[:, i, :] for i, p in enumerate("qkv")}
    mi_sb = singles.tile([P, 2, P], BF16, tag="mi", name="mi")
    m01_sb = mi_sb[:, 0, :]
    ident_sb = mi_sb[:, 1, :]

    # pre-warm the vt ring: the 129th column stays 1.0 forever (the
    # ones-feature that makes att@v emit softmax denominators in column P)
    for _ in range(3):
        vtw = vtpool.tile([P, 4, P + 1], BF16, tag="vts", name="vts")
        nc.vector.memset(vtw[:, :, P:P + 1], 1.0)

    # per-chunk assembled q/k/v (block-interleaved [dd, block, g, s]),
    # single-buffered: the pump schedule guarantees producer/consumer order.
    asm = {}

    # ---- attention pieces ----------------------------------------------
    st_e = {}       # (sb) -> masked exp tile for current chunk's scores
    st_vt = {}      # (sb) -> transposed-v SBUF tile (stage b_t -> b_m)
    st_ot = {}      # out tile in progress

    def piece_a(c, sb):
        """Scores + exp + mask for super-block sb of chunk c."""
        q2f = asm["q"].rearrange("p b g s -> p (b g s)")
        k2f = asm["k"].rearrange("p b g s -> p (b g s)")
        sT = ps_s.tile([P, 4, P], F32, tag="s", name="s")
        for j in range(4):
            sl = slice((sb * 4 + j) * P, (sb * 4 + j + 1) * P)
            nc.tensor.matmul(sT[:, j, :], lhsT=k2f[:, sl], rhs=q2f[:, sl],
                             start=True, stop=True)
        e = epool.tile([P, 4, P], BF16, tag="e", name="e")
        nc.scalar.activation(out=e[:], in_=sT[:], func=AF.Exp)
        m01_bc = m01_sb.unsqueeze(1).broadcast_to([P, 4, P])
        nc.vector.tensor_tensor(out=e[:], in0=e[:], in1=m01_bc, op=ALU.mult)
        st_e[sb] = e

    def piece_bt(c, sb):
        """Stage 1: v-transpose + PSUM->SBUF copy (with ones column)."""
        v2f = asm["v"].rearrange("p b g s -> p (b g s)")
        vt_ps = ps_vt.tile([P, 4, P], BF16, tag="vt", name="vt")
        for j in range(4):
            sl = slice((sb * 4 + j) * P, (sb * 4 + j + 1) * P)
            nc.tensor.transpose(vt_ps[:, j, :], v2f[:, sl], ident_sb)
        # vt has a 129th column preset to 1.0 (ones-feature -> denominators)
        vt = vtpool.tile([P, 4, P + 1], BF16, tag="vts", name="vts")
        nc.scalar.copy(out=vt[:, :, 0:P], in_=vt_ps[:])
        st_vt[sb] = vt

    def piece_bm(c, sb):
        """Stage 2: att@v (with fused denominators) + normalize + store."""
        if sb % 4 == 0:
            st_ot["t"] = otp.tile([P, OTB, P], BF16, tag="ot", name="ot")
        e = st_e.pop(sb)
        vt = st_vt.pop(sb)
        for h in range(2):
            o_ps = ps_o.tile([P, 2, P + 1], F32, tag="o", name="o")
            for jj in range(2):
                j = h * 2 + jj
                nc.tensor.matmul(o_ps[:, jj, :], lhsT=e[:, j, :],
                                 rhs=vt[:, j, :], start=True, stop=True)
            rec = recp.tile([P, 2, 1], F32, tag="rec", name="rec")
            nc.vector.reciprocal(out=rec[:], in_=o_ps[:, :, P:P + 1])
            rec_bc = rec[:].broadcast_to([P, 2, P])
            dst = st_ot["t"][:, (sb % 4) * 4 + h * 2:(sb % 4) * 4 + h * 2 + 2, :]
            nc.vector.tensor_tensor(out=dst, in0=o_ps[:, :, 0:P],
                                    in1=rec_bc, op=ALU.mult)
        if sb % 4 == 3:
            t0 = c * NB + (sb - 3) * 4          # first block of this out tile
            nc.gpsimd.dma_start(out=outT[:, t0:t0 + OTB, :], in_=st_ot["t"][:])

    # ---- projections with attention pieces pumped in --------------------
    def load_x(c):
        xa = xpool.tile([P, KA, CH], BF16, tag="xa", name="xa")
        nc.gpsimd.dma_start(out=xa[:], in_=xTa[c])
        xb8 = xpool.tile([P, KB, CH], FP8, tag="xb8", name="xb8")
        nc.gpsimd.dma_start(out=xb8[:], in_=xTb8[c])
        xbb = xpool.tile([P, KB, CH], BF16, tag="xbb", name="xbb")
        nc.gpsimd.dma_start(out=xbb[:], in_=xTbb[c])
        return (xa, xb8, xbb)

    def proj_slot(p, m, xt):
        xa, xb8, xbb = xt
        ps = pp_ps.tile([P, CH], F32, tag="pp", name="pp")
        if p == "v":
            w = wpool.tile([P, KT, P], BF16, tag="wt", name="wt")
            nc.sync.dma_start(out=w[:], in_=wV[m])
            for k in range(KT):
                src = xa[:, k, :] if k < KA else xbb[:, k - KA, :]
                nc.tensor.matmul(ps[:], lhsT=w[:, k, :], rhs=src,
                                 start=(k == 0), stop=(k == KT - 1))
        else:
            wa = wpa.tile([P, KA, P], BF16, tag="wa", name="wa")
            nc.sync.dma_start(out=wa[:], in_=wA[p][m])
            wb = wpb.tile([P, KB // 2, 2, P], FP8, tag="wb", name="wb")
            nc.sync.dma_start(out=wb[:], in_=wB[p][m])
            for k in range(KA):
                nc.tensor.matmul(ps[:], lhsT=wa[:, k, :], rhs=xa[:, k, :],
                                 start=(k == 0), stop=False)
            for j in range(KB // 2):
                nc.tensor.matmul(ps[:], lhsT=wb[:, j, :, :],
                                 rhs=xb8[:, 2 * j:2 * j + 2, :],
                                 start=False, stop=(j == KB // 2 - 1),
                                 perf_mode=mybir.MatmulPerfMode.DoubleRow)
        dst = asm[p][:, :, m, :]
        src = ps[:].rearrange("p (b s) -> p b s", s=8)
        nc.scalar.activation(out=dst, in_=src, func=AF.Identity,
                             bias=bias_sb[p][:, m:m + 1], scale=SCL)

    xts = {0: load_x(0)}
    for c in range(NCHUNK):
        xt = xts.pop(c)
        pending_b = []
        if c > 0:
            bt = [lambda sb=sb: piece_bt(c - 1, sb) for sb in range(NSB)]
            bm = [lambda sb=sb: piece_bm(c - 1, sb) for sb in range(NSB)]
            pending_b = [bt[0]]
            for sb in range(NSB):
                if sb + 1 < NSB:
                    pending_b.append(bt[sb + 1])
                pending_b.append(bm[sb])
        # q,k slots: pump previous chunk's att@v pieces (1 per slot)
        for i, p in enumerate("qk"):
            asm[p] = asmp.tile([P, NB, G, 8], BF16, tag=f"asm{p}",
                               name=f"asm{p}")
            for m in range(MT):
                proj_slot(p, m, xt)
                if pending_b:
                    pending_b.pop(0)()
        while pending_b:
            pending_b.pop(0)()
        if c == 0:
            nc.scalar.dma_start(out=mi_sb[:], in_=mi_dram[:])
        # v slots: pump this chunk's scores pieces (1 per slot)
        if c + 1 < NCHUNK:
            xts[c + 1] = load_x(c + 1)
        asm["v"] = asmp.tile([P, NB, G, 8], BF16, tag="asmv", name="asmv")
        for m in range(MT):
            proj_slot("v", m, xt)
            if m < NSB:
                piece_a(c, m)
    # drain: att@v of the last chunk, transposes one step ahead
    piece_bt(NCHUNK - 1, 0)
    for sb in range(NSB):
        if sb + 1 < NSB:
            piece_bt(NCHUNK - 1, sb + 1)
        piece_bm(NCHUNK - 1, sb)


_PROGRAM = None


def _build():
    global _PROGRAM
    if _PROGRAM is not None:
        return _PROGRAM
    from contextlib import ExitStack

    nc = bacc.Bacc("TRN2", target_bir_lowering=False, debug=False,
                   num_devices=N_CORES)
    with tile.TileContext(nc) as tc:
        with ExitStack() as ctx:
            _emit(nc, tc, ctx)
    nc.compile()
    _PROGRAM = nc
    return nc


def _host_inputs(x, Wq, bq, Wk, bk, Wv, bv):
    """Build the per-core input maps (host-side shard + transpose + cast)."""
    scale = 1.0 / np.sqrt(DG)
    xf = np.ascontiguousarray(x.reshape(-1, D))           # [16384, D]
    assert xf.shape[0] == N_CORES * TC

    bf = ml_dtypes.bfloat16
    e4 = ml_dtypes.float8_e4m3

    def tile_w(WT, dtype=bf):
        # [D_in, D_out] -> [MT, P, KT_part, P]: contiguous per m-tile
        kt = WT.shape[0] // P
        a = WT.reshape(kt, P, MT, P).transpose(2, 1, 0, 3)
        return np.ascontiguousarray(a).astype(dtype)

    KAF = KA * P            # 1024 bf16-contracted features
    shared = {
        "wqA": tile_w((Wq * scale).T[:KAF] * 8192),
        "wqB": tile_w((Wq * scale).T[KAF:] * 8192, e4).reshape(
            MT, P, KB // 2, 2, P),
        "wkA": tile_w(Wk.T[:KAF] * 8192),
        "wkB": tile_w(Wk.T[KAF:] * 8192, e4).reshape(MT, P, KB // 2, 2, P),
        "wvT": tile_w(Wv.T * 8192),
        "bqkv": np.ascontiguousarray(np.stack([
            (bq * scale).reshape(G, DG).T,
            bk.reshape(G, DG).T,
            bv.reshape(G, DG).T], axis=1)).astype(np.float32),
        "m01ident": np.ascontiguousarray(np.stack([
            np.kron(np.ones((G, G), dtype=np.float32),
                    np.eye(8, dtype=np.float32)),
            np.eye(P, dtype=np.float32)], axis=1)).astype(bf),
    }
    in_maps = []
    for i in range(N_CORES):
        xi = xf[i * TC:(i + 1) * TC]
        m = dict(shared)
        # [TC, D] -> tiled [NCHUNK, P, kt, CH], scaled by 16
        xs = (xi.T * 16).reshape(KT, P, NCHUNK, CH).transpose(2, 1, 0, 3)
        m["xTa"] = np.ascontiguousarray(xs[:, :, :KA]).astype(bf)
        m["xTb8"] = np.ascontiguousarray(xs[:, :, KA:]).astype(e4)
        m["xTbb"] = np.ascontiguousarray(xs[:, :, KA:]).astype(bf)
        in_maps.append(m)
    return in_maps


last_results = None


def _install_ntff_shim():
    """Provide antenv.axon_hooks if the image lacks it (profiling only)."""
    import sys
    try:
        from antenv.axon_hooks import get_axon_ntff_profile_hook  # noqa: F401
        return
    except ImportError:
        pass
    import contextlib
    import ctypes
    import types

    so_path = "/opt/axon/libaxon_pjrt.so"
    hook = None
    if os.path.exists(so_path):
        lib = ctypes.CDLL(so_path)
        if hasattr(lib, "axon_start_nrt_profile"):
            lib.axon_start_nrt_profile.argtypes = [
                ctypes.POINTER(ctypes.c_int64), ctypes.c_size_t]
            lib.axon_start_nrt_profile.restype = ctypes.c_int64
            lib.axon_stop_nrt_profile.argtypes = [ctypes.c_char_p]
            lib.axon_stop_nrt_profile.restype = ctypes.c_int64

            @contextlib.contextmanager
            def _hook(output_dir, device_ids):
                import jax
                jax.devices()
                if device_ids:
                    ids = (ctypes.c_int64 * len(device_ids))(*device_ids)
                    rc = lib.axon_start_nrt_profile(ids, len(device_ids))
                else:
                    rc = lib.axon_start_nrt_profile(None, 0)
                if rc != 0:
                    raise RuntimeError(f"axon_start_nrt_profile rc={rc}")
                try:
                    yield
                finally:
                    n = lib.axon_stop_nrt_profile(str(output_dir).encode())
                    print(f"profile: {n} file(s) written to {output_dir}")

            hook = _hook

    mod = types.ModuleType("antenv.axon_hooks")
    mod.get_axon_ntff_profile_hook = lambda: hook
    mod.set_axon_ntff_profile_hook = lambda h: None
    import antenv
    antenv.axon_hooks = mod
    sys.modules["antenv.axon_hooks"] = mod


def kernel(**inputs):
    global last_results
    nc = _build()
    in_maps = _host_inputs(**inputs)
    trace = bool(os.environ.get("BASS_TRACE"))
    if trace:
        _install_ntff_shim()
    res = run_bass_kernel_spmd(nc, in_maps, list(range(N_CORES)), trace=trace)
    last_results = res
    x = inputs["x"]
    out = np.empty((N_CORES * TC, D), dtype=np.float32)
    for i in range(N_CORES):
        o = res.results[i]["outT"].astype(np.float32)      # [P, TC/8, P]
        o = o.reshape(G, 8, TC // 8, DG).transpose(2, 1, 0, 3)
        out[i * TC:(i + 1) * TC] = o.reshape(TC, D)
    return out.reshape(x.shape)


# revision 16
# speedup vs baseline: 1.3126x; 1.0381x over previous
"""Trainium2 Bass kernel for per-token grouped attention (GQA-style).

Computation (per token t):
    q = x @ Wq.T + bq ; k = x @ Wk.T + bk ; v = x @ Wv.T + bv     (D=2048)
    reshape to (G=16 groups, d=128); scores = q_g . k_h / sqrt(d) (16x16)
    att = softmax(scores, axis=h); out = att @ v  -> (G*d,)

Sharding: data-parallel over the B*T = 16384 tokens across 8 cores
(2048 tokens/core).  Device works feature-major for the projections; the
attention emits the output token-major ([(g,s), block, dd]) and the host
unscrambles.

Device program (per core, SPMD), 4 chunks of 512 tokens:
  Projections: qT/kT/vT = W.T-tiles @ xT, bf16 matmuls with fp32 PSUM
    accumulation, bias added during the PSUM->SBUF copy (ACT), scattered
    into block-interleaved SBUF tiles [dd, block, g, s] (single-buffered).
  Attention per 8-token block b (3.01 matmul-equivalents instead of 4):
    sT = k_blk^T q_blk  (one 128x128 MM: all 64 pairwise 16x16 tiles,
         only the 8 diagonal ones survive the mask)
    e  = exp(sT) * blockdiag-mask          (ACT + DVE)
    vT = PE-transpose(v_blk)               (1 MM)
    o  = e^T @ vT   -> out^T[(g,s), dd]    (1 MM, unnormalized)
    dn = e^T @ ones -> softmax denominators (1-column MM, ~free)
    out = o * (1/dn) broadcast             (DVE, batched per super-block)
  Pump schedule (keeps PE fed, allows single-buffered q/k/v tiles):
    chunk c's q,k slots <- att@v pieces of chunk c-1
    chunk c's v slots   <- scores pieces of chunk c
    after last chunk    <- drain att@v of last chunk
"""

import os
import numpy as np
import ml_dtypes

import concourse.bass as bass
import concourse.tile as tile
from concourse import bacc, mybir
from concourse.bass_utils import run_bass_kernel_spmd

F32 = mybir.dt.float32
BF16 = mybir.dt.bfloat16
FP8 = mybir.dt.float8e4
AF = mybir.ActivationFunctionType
ALU = mybir.AluOpType

P = 128          # SBUF partitions
D = 2048         # model dim
G = 16           # groups
DG = 128         # per-group dim
N_CORES = 8
TC = 2048        # tokens per core
NCHUNK = 4
CH = TC // NCHUNK          # 512 tokens per chunk
NB = CH // 8               # 64 blocks of 8 tokens per chunk
NSB = NB // 4              # 16 super-blocks (32 tokens) per chunk
KT = D // P      # 16 contraction tiles
KA = 6           # bf16 k-tiles (features 0..767)
KB = 10          # fp8 k-tiles (features 768..2047), as KB//2 DoubleRow pairs
WAB = KA * P * 2 + KB * P  # packed qk weight bytes per partition per m-tile
MT = D // P      # 16 output-feature tiles
SCL = 2.0 ** -17  # undo the x*16 / W*8192 scaling in the PSUM->SBUF copy
OTB = 16         # blocks per output tile (128 tokens)


def _emit(nc, tc, ctx):
    # ---- DRAM I/O -------------------------------------------------------
    xTa = nc.dram_tensor("xTa", [NCHUNK, P, KA, CH], BF16,
                         kind="ExternalInput").ap()
    xTb8 = nc.dram_tensor("xTb8", [NCHUNK, P, KB, CH], FP8,
                          kind="ExternalInput").ap()
    xTbb = nc.dram_tensor("xTbb", [NCHUNK, P, KB, CH], BF16,
                          kind="ExternalInput").ap()
    wQK = {
        p: nc.dram_tensor(f"w{p}AB", [MT, P, WAB], mybir.dt.uint8,
                          kind="ExternalInput").ap()
        for p in "qk"
    }
    wV = nc.dram_tensor("wvT", [MT, P, KT, P], BF16,
                        kind="ExternalInput").ap()
    b_dram = nc.dram_tensor("bqkv", [P, 3, G], F32, kind="ExternalInput").ap()
    mi_dram = nc.dram_tensor("m01ident", [P, 2, P], BF16,
                             kind="ExternalInput").ap()
    outT = nc.dram_tensor("outT", [P, TC // 8, P], BF16,
                          kind="ExternalOutput").ap()

    # ---- pools ----------------------------------------------------------
    singles = ctx.enter_context(tc.tile_pool(name="singles", bufs=1))
    xpool = ctx.enter_context(tc.tile_pool(name="xpool", bufs=2))
    wpool = ctx.enter_context(tc.tile_pool(name="wpool", bufs=6))
    wpab = ctx.enter_context(tc.tile_pool(name="wpab", bufs=6))
    asmp = ctx.enter_context(tc.tile_pool(name="asmp", bufs=1))
    epool = ctx.enter_context(tc.tile_pool(name="epool", bufs=NSB + 2))
    vtpool = ctx.enter_context(tc.tile_pool(name="vtpool", bufs=3))
    recp = ctx.enter_context(tc.tile_pool(name="recp", bufs=3))
    otp = ctx.enter_context(tc.tile_pool(name="otp", bufs=2))

    pp_ps = ctx.enter_context(tc.tile_pool(name="pp_ps", bufs=3, space="PSUM"))
    ps_s = ctx.enter_context(tc.tile_pool(name="ps_s", bufs=1, space="PSUM"))
    ps_vt = ctx.enter_context(tc.tile_pool(name="ps_vt", bufs=2, space="PSUM"))
    ps_o = ctx.enter_context(tc.tile_pool(name="ps_o", bufs=2, space="PSUM"))

    # ---- constants (keep the SP/sync queue free for weight tiles) -------
    ball = singles.tile([P, 3, G], F32, tag="bias", name="bias")
    nc.scalar.dma_start(out=ball[:], in_=b_dram[:])
    bias_sb = {p: ball[:, i, :] for i, p in enumerate("qkv")}
    mi_sb = singles.tile([P, 2, P], BF16, tag="mi", name="mi")
    m01_sb = mi_sb[:, 0, :]
    ident_sb = mi_sb[:, 1, :]

    # pre-warm the vt ring: the 129th column stays 1.0 forever (the
    # ones-feature that makes att@v emit softmax denominators in column P)
    for _ in range(3):
        vtw = vtpool.tile([P, 4, P + 1], BF16, tag="vts", name="vts")
        nc.vector.memset(vtw[:, :, P:P + 1], 1.0)

    # per-chunk assembled q/k/v (block-interleaved [dd, block, g, s]),
    # single-buffered: the pump schedule guarantees producer/consumer order.
    asm = {}

    # ---- attention pieces ----------------------------------------------
    st_e = {}       # (sb) -> masked exp tile for current chunk's scores
    st_vt = {}      # (sb) -> transposed-v SBUF tile (stage b_t -> b_m)
    st_ot = {}      # out tile in progress

    def piece_a(c, sb):
        """Scores + exp + mask for super-block sb of chunk c."""
        q2f = asm["q"].rearrange("p b g s -> p (b g s)")
        k2f = asm["k"].rearrange("p b g s -> p (b g s)")
        sT = ps_s.tile([P, 4, P], F32, tag="s", name="s")
        for j in range(4):
            sl = slice((sb * 4 + j) * P, (sb * 4 + j + 1) * P)
            nc.tensor.matmul(sT[:, j, :], lhsT=k2f[:, sl], rhs=q2f[:, sl],
                             start=True, stop=True)
        e = epool.tile([P, 4, P], BF16, tag="e", name="e")
        nc.scalar.activation(out=e[:], in_=sT[:], func=AF.Exp)
        m01_bc = m01_sb.unsqueeze(1).broadcast_to([P, 4, P])
        nc.vector.tensor_tensor(out=e[:], in0=e[:], in1=m01_bc, op=ALU.mult)
        st_e[sb] = e

    def piece_bt(c, sb):
        """Stage 1: v-transpose + PSUM->SBUF copy (with ones column)."""
        v2f = asm["v"].rearrange("p b g s -> p (b g s)")
        vt_ps = ps_vt.tile([P, 4, P], BF16, tag="vt", name="vt")
        for j in range(4):
            sl = slice((sb * 4 + j) * P, (sb * 4 + j + 1) * P)
            nc.tensor.transpose(vt_ps[:, j, :], v2f[:, sl], ident_sb)
        # vt has a 129th column preset to 1.0 (ones-feature -> denominators)
        vt = vtpool.tile([P, 4, P + 1], BF16, tag="vts", name="vts")
        nc.scalar.copy(out=vt[:, :, 0:P], in_=vt_ps[:])
        st_vt[sb] = vt

    def piece_bm(c, sb):
        """Stage 2: att@v (with fused denominators) + normalize + store."""
        if sb % 4 == 0:
            st_ot["t"] = otp.tile([P, OTB, P], BF16, tag="ot", name="ot")
        e = st_e.pop(sb)
        vt = st_vt.pop(sb)
        for h in range(2):
            o_ps = ps_o.tile([P, 2, P + 1], F32, tag="o", name="o")
            for jj in range(2):
                j = h * 2 + jj
                nc.tensor.matmul(o_ps[:, jj, :], lhsT=e[:, j, :],
                                 rhs=vt[:, j, :], start=True, stop=True)
            rec = recp.tile([P, 2, 1], F32, tag="rec", name="rec")
            nc.vector.reciprocal(out=rec[:], in_=o_ps[:, :, P:P + 1])
            rec_bc = rec[:].broadcast_to([P, 2, P])
            dst = st_ot["t"][:, (sb % 4) * 4 + h * 2:(sb % 4) * 4 + h * 2 + 2, :]
            nc.vector.tensor_tensor(out=dst, in0=o_ps[:, :, 0:P],
                                    in1=rec_bc, op=ALU.mult)
        if sb % 4 == 3:
            t0 = c * NB + (sb - 3) * 4          # first block of this out tile
            nc.gpsimd.dma_start(out=outT[:, t0:t0 + OTB, :], in_=st_ot["t"][:])

    # ---- projections with attention pieces pumped in --------------------
    def load_x(c):
        xa = xpool.tile([P, KA, CH], BF16, tag="xa", name="xa")
        nc.gpsimd.dma_start(out=xa[:], in_=xTa[c])
        xb8 = xpool.tile([P, KB, CH], FP8, tag="xb8", name="xb8")
        nc.gpsimd.dma_start(out=xb8[:], in_=xTb8[c])
        xbb = xpool.tile([P, KB, CH], BF16, tag="xbb", name="xbb")
        nc.gpsimd.dma_start(out=xbb[:], in_=xTbb[c])
        return (xa, xb8, xbb)

    def proj_slot(p, m, xt):
        xa, xb8, xbb = xt
        ps = pp_ps.tile([P, CH], F32, tag="pp", name="pp")
        if p == "v":
            w = wpool.tile([P, KT, P], BF16, tag="wt", name="wt")
            nc.sync.dma_start(out=w[:], in_=wV[m])
            for k in range(KT):
                src = xa[:, k, :] if k < KA else xbb[:, k - KA, :]
                nc.tensor.matmul(ps[:], lhsT=w[:, k, :], rhs=src,
                                 start=(k == 0), stop=(k == KT - 1))
        else:
            w = wpab.tile([P, WAB], mybir.dt.uint8, tag="wab", name="wab")
            nc.sync.dma_start(out=w[:], in_=wQK[p][m])
            wa = w[:, 0:KA * P * 2].bitcast(BF16).rearrange(
                "p (k o) -> p k o", o=P)
            wb = w[:, KA * P * 2:WAB].bitcast(FP8).rearrange(
                "p (j t o) -> p j t o", t=2, o=P)
            for k in range(KA):
                nc.tensor.matmul(ps[:], lhsT=wa[:, k, :], rhs=xa[:, k, :],
                                 start=(k == 0), stop=False)
            for j in range(KB // 2):
                nc.tensor.matmul(ps[:], lhsT=wb[:, j, :, :],
                                 rhs=xb8[:, 2 * j:2 * j + 2, :],
                                 start=False, stop=(j == KB // 2 - 1),
                                 perf_mode=mybir.MatmulPerfMode.DoubleRow)
        dst = asm[p][:, :, m, :]
        src = ps[:].rearrange("p (b s) -> p b s", s=8)
        nc.scalar.activation(out=dst, in_=src, func=AF.Identity,
                             bias=bias_sb[p][:, m:m + 1], scale=SCL)

    xts = {0: load_x(0)}
    for c in range(NCHUNK):
        xt = xts.pop(c)
        pending_b = []
        if c > 0:
            bt = [lambda sb=sb: piece_bt(c - 1, sb) for sb in range(NSB)]
            bm = [lambda sb=sb: piece_bm(c - 1, sb) for sb in range(NSB)]
            pending_b = [bt[0], bt[1]]
            for sb in range(NSB):
                if sb + 2 < NSB:
                    pending_b.append(bt[sb + 2])
                pending_b.append(bm[sb])
        # q,k slots: pump previous chunk's att@v pieces (1 per slot)
        for i, p in enumerate("qk"):
            asm[p] = asmp.tile([P, NB, G, 8], BF16, tag=f"asm{p}",
                               name=f"asm{p}")
            for m in range(MT):
                proj_slot(p, m, xt)
                if pending_b:
                    pending_b.pop(0)()
        while pending_b:
            pending_b.pop(0)()
        if c == 0:
            nc.scalar.dma_start(out=mi_sb[:], in_=mi_dram[:])
        # v slots: pump this chunk's scores pieces (1 per slot)
        if c + 1 < NCHUNK:
            xts[c + 1] = load_x(c + 1)
        asm["v"] = asmp.tile([P, NB, G, 8], BF16, tag="asmv", name="asmv")
        for m in range(MT):
            proj_slot("v", m, xt)
            if m < NSB:
                piece_a(c, m)
    # drain: att@v of the last chunk, transposes two steps ahead
    piece_bt(NCHUNK - 1, 0)
    piece_bt(NCHUNK - 1, 1)
    for sb in range(NSB):
        if sb + 2 < NSB:
            piece_bt(NCHUNK - 1, sb + 2)
        piece_bm(NCHUNK - 1, sb)


_PROGRAM = None


def _build():
    global _PROGRAM
    if _PROGRAM is not None:
        return _PROGRAM
    from contextlib import ExitStack

    nc = bacc.Bacc("TRN2", target_bir_lowering=False, debug=False,
                   num_devices=N_CORES)
    with tile.TileContext(nc) as tc:
        with ExitStack() as ctx:
            _emit(nc, tc, ctx)
    nc.compile()
    _PROGRAM = nc
    return nc


def _host_inputs(x, Wq, bq, Wk, bk, Wv, bv):
    """Build the per-core input maps (host-side shard + transpose + cast)."""
    scale = 1.0 / np.sqrt(DG)
    xf = np.ascontiguousarray(x.reshape(-1, D))           # [16384, D]
    assert xf.shape[0] == N_CORES * TC

    bf = ml_dtypes.bfloat16
    e4 = ml_dtypes.float8_e4m3

    def tile_w(WT, dtype=bf):
        # [D_in, D_out] -> [MT, P, KT_part, P]: contiguous per m-tile
        kt = WT.shape[0] // P
        a = WT.reshape(kt, P, MT, P).transpose(2, 1, 0, 3)
        return np.ascontiguousarray(a).astype(dtype)

    KAF = KA * P            # bf16-contracted features

    def pack_qk(WT):
        a = tile_w(WT[:KAF] * 8192)                      # [MT,P,KA,P] bf16
        b = tile_w(WT[KAF:] * 8192, e4)                  # [MT,P,KB,P] fp8
        ab = np.concatenate([
            a.view(np.uint8).reshape(MT, P, -1),
            b.view(np.uint8).reshape(MT, P, -1)], axis=2)
        return np.ascontiguousarray(ab)

    shared = {
        "wqAB": pack_qk((Wq * scale).T),
        "wkAB": pack_qk(Wk.T),
        "wvT": tile_w(Wv.T * 8192),
        "bqkv": np.ascontiguousarray(np.stack([
            (bq * scale).reshape(G, DG).T,
            bk.reshape(G, DG).T,
            bv.reshape(G, DG).T], axis=1)).astype(np.float32),
        "m01ident": np.ascontiguousarray(np.stack([
            np.kron(np.ones((G, G), dtype=np.float32),
                    np.eye(8, dtype=np.float32)),
            np.eye(P, dtype=np.float32)], axis=1)).astype(bf),
    }
    in_maps = []
    for i in range(N_CORES):
        xi = xf[i * TC:(i + 1) * TC]
        m = dict(shared)
        # [TC, D] -> tiled [NCHUNK, P, kt, CH], scaled by 16
        xs = (xi.T * 16).reshape(KT, P, NCHUNK, CH).transpose(2, 1, 0, 3)
        m["xTa"] = np.ascontiguousarray(xs[:, :, :KA]).astype(bf)
        m["xTb8"] = np.ascontiguousarray(xs[:, :, KA:]).astype(e4)
        m["xTbb"] = np.ascontiguousarray(xs[:, :, KA:]).astype(bf)
        in_maps.append(m)
    return in_maps


last_results = None


def _install_ntff_shim():
    """Provide antenv.axon_hooks if the image lacks it (profiling only)."""
    import sys
    try:
        from antenv.axon_hooks import get_axon_ntff_profile_hook  # noqa: F401
        return
    except ImportError:
        pass
    import contextlib
    import ctypes
    import types

    so_path = "/opt/axon/libaxon_pjrt.so"
    hook = None
    if os.path.exists(so_path):
        lib = ctypes.CDLL(so_path)
        if hasattr(lib, "axon_start_nrt_profile"):
            lib.axon_start_nrt_profile.argtypes = [
                ctypes.POINTER(ctypes.c_int64), ctypes.c_size_t]
            lib.axon_start_nrt_profile.restype = ctypes.c_int64
            lib.axon_stop_nrt_profile.argtypes = [ctypes.c_char_p]
            lib.axon_stop_nrt_profile.restype = ctypes.c_int64

            @contextlib.contextmanager
            def _hook(output_dir, device_ids):
                import jax
                jax.devices()
                if device_ids:
                    ids = (ctypes.c_int64 * len(device_ids))(*device_ids)
                    rc = lib.axon_start_nrt_profile(ids, len(device_ids))
                else:
                    rc = lib.axon_start_nrt_profile(None, 0)
                if rc != 0:
                    raise RuntimeError(f"axon_start_nrt_profile rc={rc}")
                try:
                    yield
                finally:
                    n = lib.axon_stop_nrt_profile(str(output_dir).encode())
                    print(f"profile: {n} file(s) written to {output_dir}")

            hook = _hook

    mod = types.ModuleType("antenv.axon_hooks")
    mod.get_axon_ntff_profile_hook = lambda: hook
    mod.set_axon_ntff_profile_hook = lambda h: None
    import antenv
    antenv.axon_hooks = mod
    sys.modules["antenv.axon_hooks"] = mod


def kernel(**inputs):
    global last_results
    nc = _build()
    in_maps = _host_inputs(**inputs)
    trace = bool(os.environ.get("BASS_TRACE"))
    if trace:
        _install_ntff_shim()
    res = run_bass_kernel_spmd(nc, in_maps, list(range(N_CORES)), trace=trace)
    last_results = res
    x = inputs["x"]
    out = np.empty((N_CORES * TC, D), dtype=np.float32)
    for i in range(N_CORES):
        o = res.results[i]["outT"].astype(np.float32)      # [P, TC/8, P]
        o = o.reshape(G, 8, TC // 8, DG).transpose(2, 1, 0, 3)
        out[i * TC:(i + 1) * TC] = o.reshape(TC, D)
    return out.reshape(x.shape)


# revision 17
# speedup vs baseline: 1.3224x; 1.0075x over previous
"""Trainium2 Bass kernel for per-token grouped attention (GQA-style).

Computation (per token t):
    q = x @ Wq.T + bq ; k = x @ Wk.T + bk ; v = x @ Wv.T + bv     (D=2048)
    reshape to (G=16 groups, d=128); scores = q_g . k_h / sqrt(d) (16x16)
    att = softmax(scores, axis=h); out = att @ v  -> (G*d,)

Sharding: data-parallel over the B*T = 16384 tokens across 8 cores
(2048 tokens/core).  Device works feature-major for the projections; the
attention emits the output token-major ([(g,s), block, dd]) and the host
unscrambles.

Device program (per core, SPMD), 4 chunks of 512 tokens:
  Projections: qT/kT/vT = W.T-tiles @ xT, bf16 matmuls with fp32 PSUM
    accumulation, bias added during the PSUM->SBUF copy (ACT), scattered
    into block-interleaved SBUF tiles [dd, block, g, s] (single-buffered).
  Attention per 8-token block b (3.01 matmul-equivalents instead of 4):
    sT = k_blk^T q_blk  (one 128x128 MM: all 64 pairwise 16x16 tiles,
         only the 8 diagonal ones survive the mask)
    e  = exp(sT) * blockdiag-mask          (ACT + DVE)
    vT = PE-transpose(v_blk)               (1 MM)
    o  = e^T @ vT   -> out^T[(g,s), dd]    (1 MM, unnormalized)
    dn = e^T @ ones -> softmax denominators (1-column MM, ~free)
    out = o * (1/dn) broadcast             (DVE, batched per super-block)
  Pump schedule (keeps PE fed, allows single-buffered q/k/v tiles):
    chunk c's q,k slots <- att@v pieces of chunk c-1
    chunk c's v slots   <- scores pieces of chunk c
    after last chunk    <- drain att@v of last chunk
"""

import os
import numpy as np
import ml_dtypes

import concourse.bass as bass
import concourse.tile as tile
from concourse import bacc, mybir
from concourse.bass_utils import run_bass_kernel_spmd

F32 = mybir.dt.float32
BF16 = mybir.dt.bfloat16
FP8 = mybir.dt.float8e4
AF = mybir.ActivationFunctionType
ALU = mybir.AluOpType

P = 128          # SBUF partitions
D = 2048         # model dim
G = 16           # groups
DG = 128         # per-group dim
N_CORES = 8
TC = 2048        # tokens per core
NCHUNK = 4
CH = TC // NCHUNK          # 512 tokens per chunk
NB = CH // 8               # 64 blocks of 8 tokens per chunk
NSB = NB // 4              # 16 super-blocks (32 tokens) per chunk
KT = D // P      # 16 contraction tiles
KA = 6           # bf16 k-tiles (features 0..767)
KB = 10          # fp8 k-tiles (features 768..2047), as KB//2 DoubleRow pairs
WAB = KA * P * 2 + KB * P  # packed qk weight bytes per partition per m-tile
MT = D // P      # 16 output-feature tiles
SCL = 2.0 ** -17  # undo the x*16 / W*8192 scaling in the PSUM->SBUF copy
OTB = 16         # blocks per output tile (128 tokens)


def _emit(nc, tc, ctx):
    # ---- DRAM I/O -------------------------------------------------------
    xTa = nc.dram_tensor("xTa", [NCHUNK, P, KA, CH], BF16,
                         kind="ExternalInput").ap()
    xTb8 = nc.dram_tensor("xTb8", [NCHUNK, P, KB, CH], FP8,
                          kind="ExternalInput").ap()
    xTbb = nc.dram_tensor("xTbb", [NCHUNK, P, KB, CH], BF16,
                          kind="ExternalInput").ap()
    wQK = {
        p: nc.dram_tensor(f"w{p}AB", [MT // 2, P, 2, WAB], mybir.dt.uint8,
                          kind="ExternalInput").ap()
        for p in "qk"
    }
    wV = nc.dram_tensor("wvT", [MT // 2, P, 2, KT, P], BF16,
                        kind="ExternalInput").ap()
    b_dram = nc.dram_tensor("bqkv", [P, 3, G], F32, kind="ExternalInput").ap()
    mi_dram = nc.dram_tensor("m01ident", [P, 2, P], BF16,
                             kind="ExternalInput").ap()
    outT = nc.dram_tensor("outT", [P, TC // 8, P], BF16,
                          kind="ExternalOutput").ap()

    # ---- pools ----------------------------------------------------------
    singles = ctx.enter_context(tc.tile_pool(name="singles", bufs=1))
    xpool = ctx.enter_context(tc.tile_pool(name="xpool", bufs=2))
    wpool = ctx.enter_context(tc.tile_pool(name="wpool", bufs=6))
    wpab = ctx.enter_context(tc.tile_pool(name="wpab", bufs=6))
    asmp = ctx.enter_context(tc.tile_pool(name="asmp", bufs=1))
    epool = ctx.enter_context(tc.tile_pool(name="epool", bufs=NSB + 2))
    vtpool = ctx.enter_context(tc.tile_pool(name="vtpool", bufs=3))
    recp = ctx.enter_context(tc.tile_pool(name="recp", bufs=3))
    otp = ctx.enter_context(tc.tile_pool(name="otp", bufs=2))

    pp_ps = ctx.enter_context(tc.tile_pool(name="pp_ps", bufs=2, space="PSUM"))
    ps_s = ctx.enter_context(tc.tile_pool(name="ps_s", bufs=1, space="PSUM"))
    ps_vt = ctx.enter_context(tc.tile_pool(name="ps_vt", bufs=2, space="PSUM"))
    ps_o = ctx.enter_context(tc.tile_pool(name="ps_o", bufs=3, space="PSUM"))

    # ---- constants (keep the SP/sync queue free for weight tiles) -------
    ball = singles.tile([P, 3, G], F32, tag="bias", name="bias")
    nc.scalar.dma_start(out=ball[:], in_=b_dram[:])
    bias_sb = {p: ball[:, i, :] for i, p in enumerate("qkv")}
    mi_sb = singles.tile([P, 2, P], BF16, tag="mi", name="mi")
    m01_sb = mi_sb[:, 0, :]
    ident_sb = mi_sb[:, 1, :]

    # pre-warm the vt ring: the 129th column stays 1.0 forever (the
    # ones-feature that makes att@v emit softmax denominators in column P)
    for _ in range(3):
        vtw = vtpool.tile([P, 4, P + 1], BF16, tag="vts", name="vts")
        nc.vector.memset(vtw[:, :, P:P + 1], 1.0)

    # per-chunk assembled q/k/v (block-interleaved [dd, block, g, s]),
    # single-buffered: the pump schedule guarantees producer/consumer order.
    asm = {}

    # ---- attention pieces ----------------------------------------------
    st_e = {}       # (sb) -> masked exp tile for current chunk's scores
    st_vt = {}      # (sb) -> transposed-v SBUF tile (stage b_t -> b_m)
    st_ot = {}      # out tile in progress

    def piece_a(c, sb):
        """Scores + exp + mask for super-block sb of chunk c."""
        q2f = asm["q"].rearrange("p b g s -> p (b g s)")
        k2f = asm["k"].rearrange("p b g s -> p (b g s)")
        sT = ps_s.tile([P, 4, P], F32, tag="s", name="s")
        for j in range(4):
            sl = slice((sb * 4 + j) * P, (sb * 4 + j + 1) * P)
            nc.tensor.matmul(sT[:, j, :], lhsT=k2f[:, sl], rhs=q2f[:, sl],
                             start=True, stop=True)
        e = epool.tile([P, 4, P], BF16, tag="e", name="e")
        nc.scalar.activation(out=e[:], in_=sT[:], func=AF.Exp)
        m01_bc = m01_sb.unsqueeze(1).broadcast_to([P, 4, P])
        nc.vector.tensor_tensor(out=e[:], in0=e[:], in1=m01_bc, op=ALU.mult)
        st_e[sb] = e

    def piece_bt(c, sb):
        """Stage 1: v-transpose + PSUM->SBUF copy (with ones column)."""
        v2f = asm["v"].rearrange("p b g s -> p (b g s)")
        vt_ps = ps_vt.tile([P, 4, P], BF16, tag="vt", name="vt")
        for j in range(4):
            sl = slice((sb * 4 + j) * P, (sb * 4 + j + 1) * P)
            nc.tensor.transpose(vt_ps[:, j, :], v2f[:, sl], ident_sb)
        # vt has a 129th column preset to 1.0 (ones-feature -> denominators)
        vt = vtpool.tile([P, 4, P + 1], BF16, tag="vts", name="vts")
        nc.scalar.copy(out=vt[:, :, 0:P], in_=vt_ps[:])
        st_vt[sb] = vt

    def piece_bm(c, sb):
        """Stage 2: att@v (with fused denominators) + normalize + store."""
        if sb % 4 == 0:
            st_ot["t"] = otp.tile([P, OTB, P], BF16, tag="ot", name="ot")
        e = st_e.pop(sb)
        vt = st_vt.pop(sb)
        for h in range(2):
            o_ps = ps_o.tile([P, 2, P + 1], F32, tag="o", name="o")
            for jj in range(2):
                j = h * 2 + jj
                nc.tensor.matmul(o_ps[:, jj, :], lhsT=e[:, j, :],
                                 rhs=vt[:, j, :], start=True, stop=True)
            rec = recp.tile([P, 2, 1], F32, tag="rec", name="rec")
            nc.vector.reciprocal(out=rec[:], in_=o_ps[:, :, P:P + 1])
            rec_bc = rec[:].broadcast_to([P, 2, P])
            dst = st_ot["t"][:, (sb % 4) * 4 + h * 2:(sb % 4) * 4 + h * 2 + 2, :]
            nc.vector.tensor_tensor(out=dst, in0=o_ps[:, :, 0:P],
                                    in1=rec_bc, op=ALU.mult)
        if sb % 4 == 3:
            t0 = c * NB + (sb - 3) * 4          # first block of this out tile
            nc.gpsimd.dma_start(out=outT[:, t0:t0 + OTB, :], in_=st_ot["t"][:])

    # ---- projections with attention pieces pumped in --------------------
    def load_x(c):
        xa = xpool.tile([P, KA, CH], BF16, tag="xa", name="xa")
        nc.gpsimd.dma_start(out=xa[:], in_=xTa[c])
        xb8 = xpool.tile([P, KB, CH], FP8, tag="xb8", name="xb8")
        nc.gpsimd.dma_start(out=xb8[:], in_=xTb8[c])
        xbb = xpool.tile([P, KB, CH], BF16, tag="xbb", name="xbb")
        nc.gpsimd.dma_start(out=xbb[:], in_=xTbb[c])
        return (xa, xb8, xbb)

    def proj_pair(p, mp, xt):
        """Two m-tiles per slot, one weight DMA -> one sem wait."""
        xa, xb8, xbb = xt
        if p == "v":
            w2 = wpool.tile([P, 2, KT, P], BF16, tag="wt", name="wt")
            nc.sync.dma_start(out=w2[:], in_=wV[mp])
        else:
            w2 = wpab.tile([P, 2, WAB], mybir.dt.uint8, tag="wab", name="wab")
            nc.sync.dma_start(out=w2[:], in_=wQK[p][mp])
        for i in range(2):
            m = 2 * mp + i
            ps = pp_ps.tile([P, CH], F32, tag="pp", name="pp")
            if p == "v":
                w = w2[:, i]
                for k in range(KT):
                    src = xa[:, k, :] if k < KA else xbb[:, k - KA, :]
                    nc.tensor.matmul(ps[:], lhsT=w[:, k, :], rhs=src,
                                     start=(k == 0), stop=(k == KT - 1))
            else:
                wa = w2[:, i, 0:KA * P * 2].bitcast(BF16).rearrange(
                    "p (k o) -> p k o", o=P)
                wb = w2[:, i, KA * P * 2:WAB].bitcast(FP8).rearrange(
                    "p (j t o) -> p j t o", t=2, o=P)
                for k in range(KA):
                    nc.tensor.matmul(ps[:], lhsT=wa[:, k, :], rhs=xa[:, k, :],
                                     start=(k == 0), stop=False)
                for j in range(KB // 2):
                    nc.tensor.matmul(ps[:], lhsT=wb[:, j, :, :],
                                     rhs=xb8[:, 2 * j:2 * j + 2, :],
                                     start=False, stop=(j == KB // 2 - 1),
                                     perf_mode=mybir.MatmulPerfMode.DoubleRow)
            dst = asm[p][:, :, m, :]
            src = ps[:].rearrange("p (b s) -> p b s", s=8)
            nc.scalar.activation(out=dst, in_=src, func=AF.Identity,
                                 bias=bias_sb[p][:, m:m + 1], scale=SCL)

    xts = {0: load_x(0)}
    for c in range(NCHUNK):
        xt = xts.pop(c)
        pending_b = []
        if c > 0:
            bt = [lambda sb=sb: piece_bt(c - 1, sb) for sb in range(NSB)]
            bm = [lambda sb=sb: piece_bm(c - 1, sb) for sb in range(NSB)]
            pending_b = [bt[0], bt[1]]
            for sb in range(NSB):
                if sb + 2 < NSB:
                    pending_b.append(bt[sb + 2])
                pending_b.append(bm[sb])
        # q,k slots: pump previous chunk's att@v pieces (2 per slot)
        for i, p in enumerate("qk"):
            asm[p] = asmp.tile([P, NB, G, 8], BF16, tag=f"asm{p}",
                               name=f"asm{p}")
            for mp in range(MT // 2):
                proj_pair(p, mp, xt)
                for _ in range(2):
                    if pending_b:
                        pending_b.pop(0)()
        while pending_b:
            pending_b.pop(0)()
        if c == 0:
            nc.scalar.dma_start(out=mi_sb[:], in_=mi_dram[:])
        # v slots: pump this chunk's scores pieces (1 per slot)
        if c + 1 < NCHUNK:
            xts[c + 1] = load_x(c + 1)
        asm["v"] = asmp.tile([P, NB, G, 8], BF16, tag="asmv", name="asmv")
        for mp in range(MT // 2):
            proj_pair("v", mp, xt)
            piece_a(c, 2 * mp)
            piece_a(c, 2 * mp + 1)
    # drain: att@v of the last chunk, transposes two steps ahead
    piece_bt(NCHUNK - 1, 0)
    piece_bt(NCHUNK - 1, 1)
    for sb in range(NSB):
        if sb + 2 < NSB:
            piece_bt(NCHUNK - 1, sb + 2)
        piece_bm(NCHUNK - 1, sb)


_PROGRAM = None


def _build():
    global _PROGRAM
    if _PROGRAM is not None:
        return _PROGRAM
    from contextlib import ExitStack

    nc = bacc.Bacc("TRN2", target_bir_lowering=False, debug=False,
                   num_devices=N_CORES)
    with tile.TileContext(nc) as tc:
        with ExitStack() as ctx:
            _emit(nc, tc, ctx)
    nc.compile()
    _PROGRAM = nc
    return nc


def _host_inputs(x, Wq, bq, Wk, bk, Wv, bv):
    """Build the per-core input maps (host-side shard + transpose + cast)."""
    scale = 1.0 / np.sqrt(DG)
    xf = np.ascontiguousarray(x.reshape(-1, D))           # [16384, D]
    assert xf.shape[0] == N_CORES * TC

    bf = ml_dtypes.bfloat16
    e4 = ml_dtypes.float8_e4m3

    def tile_w(WT, dtype=bf):
        # [D_in, D_out] -> [MT, P, KT_part, P]: contiguous per m-tile
        kt = WT.shape[0] // P
        a = WT.reshape(kt, P, MT, P).transpose(2, 1, 0, 3)
        return np.ascontiguousarray(a).astype(dtype)

    KAF = KA * P            # bf16-contracted features

    def pack_qk(WT):
        a = tile_w(WT[:KAF] * 8192)                      # [MT,P,KA,P] bf16
        b = tile_w(WT[KAF:] * 8192, e4)                  # [MT,P,KB,P] fp8
        ab = np.concatenate([
            a.view(np.uint8).reshape(MT, P, -1),
            b.view(np.uint8).reshape(MT, P, -1)], axis=2)
        # pair m-tiles: [MT,P,WAB] -> [MT/2, P, 2, WAB]
        ab = ab.reshape(MT // 2, 2, P, WAB).transpose(0, 2, 1, 3)
        return np.ascontiguousarray(ab)

    shared = {
        "wqAB": pack_qk((Wq * scale).T),
        "wkAB": pack_qk(Wk.T),
        "wvT": np.ascontiguousarray(
            tile_w(Wv.T * 8192).reshape(MT // 2, 2, P, KT, P)
            .transpose(0, 2, 1, 3, 4)),
        "bqkv": np.ascontiguousarray(np.stack([
            (bq * scale).reshape(G, DG).T,
            bk.reshape(G, DG).T,
            bv.reshape(G, DG).T], axis=1)).astype(np.float32),
        "m01ident": np.ascontiguousarray(np.stack([
            np.kron(np.ones((G, G), dtype=np.float32),
                    np.eye(8, dtype=np.float32)),
            np.eye(P, dtype=np.float32)], axis=1)).astype(bf),
    }
    in_maps = []
    for i in range(N_CORES):
        xi = xf[i * TC:(i + 1) * TC]
        m = dict(shared)
        # [TC, D] -> tiled [NCHUNK, P, kt, CH], scaled by 16
        xs = (xi.T * 16).reshape(KT, P, NCHUNK, CH).transpose(2, 1, 0, 3)
        m["xTa"] = np.ascontiguousarray(xs[:, :, :KA]).astype(bf)
        m["xTb8"] = np.ascontiguousarray(xs[:, :, KA:]).astype(e4)
        m["xTbb"] = np.ascontiguousarray(xs[:, :, KA:]).astype(bf)
        in_maps.append(m)
    return in_maps


last_results = None


def _install_ntff_shim():
    """Provide antenv.axon_hooks if the image lacks it (profiling only)."""
    import sys
    try:
        from antenv.axon_hooks import get_axon_ntff_profile_hook  # noqa: F401
        return
    except ImportError:
        pass
    import contextlib
    import ctypes
    import types

    so_path = "/opt/axon/libaxon_pjrt.so"
    hook = None
    if os.path.exists(so_path):
        lib = ctypes.CDLL(so_path)
        if hasattr(lib, "axon_start_nrt_profile"):
            lib.axon_start_nrt_profile.argtypes = [
                ctypes.POINTER(ctypes.c_int64), ctypes.c_size_t]
            lib.axon_start_nrt_profile.restype = ctypes.c_int64
            lib.axon_stop_nrt_profile.argtypes = [ctypes.c_char_p]
            lib.axon_stop_nrt_profile.restype = ctypes.c_int64

            @contextlib.contextmanager
            def _hook(output_dir, device_ids):
                import jax
                jax.devices()
                if device_ids:
                    ids = (ctypes.c_int64 * len(device_ids))(*device_ids)
                    rc = lib.axon_start_nrt_profile(ids, len(device_ids))
                else:
                    rc = lib.axon_start_nrt_profile(None, 0)
                if rc != 0:
                    raise RuntimeError(f"axon_start_nrt_profile rc={rc}")
                try:
                    yield
                finally:
                    n = lib.axon_stop_nrt_profile(str(output_dir).encode())
                    print(f"profile: {n} file(s) written to {output_dir}")

            hook = _hook

    mod = types.ModuleType("antenv.axon_hooks")
    mod.get_axon_ntff_profile_hook = lambda: hook
    mod.set_axon_ntff_profile_hook = lambda h: None
    import antenv
    antenv.axon_hooks = mod
    sys.modules["antenv.axon_hooks"] = mod


def kernel(**inputs):
    global last_results
    nc = _build()
    in_maps = _host_inputs(**inputs)
    trace = bool(os.environ.get("BASS_TRACE"))
    if trace:
        _install_ntff_shim()
    res = run_bass_kernel_spmd(nc, in_maps, list(range(N_CORES)), trace=trace)
    last_results = res
    x = inputs["x"]
    out = np.empty((N_CORES * TC, D), dtype=np.float32)
    for i in range(N_CORES):
        o = res.results[i]["outT"].astype(np.float32)      # [P, TC/8, P]
        o = o.reshape(G, 8, TC // 8, DG).transpose(2, 1, 0, 3)
        out[i * TC:(i + 1) * TC] = o.reshape(TC, D)
    return out.reshape(x.shape)
